# revision 1
# baseline (speedup 1.0000x reference)
# GCN message-passing kernel for Trainium2 (8 NeuronCores, MPMD).
#
# Math (PyG GCNConv x2 + per-graph MLP readout):
#   A_norm[c,r] = dinv[c] * ew * dinv[r]   (incl. self loops w=1),  dinv = rsqrt(deg)
#   h1 = leaky_relu(A_norm @ x  @ W1 + b1)
#   h2 =            A_norm @ h1 @ W2 + b2
#   z  = reshape(h2, [B, 22*128]);  MLP;  out = tanh(z)*90 + 150
#
# Edges are uniformly random over all nodes (the graphs are NOT closed
# components), so conv2 needs a real device-side gather of h1 rows.
#
# Device plan (3 launches, nodes sharded 22528/core contiguously):
#   L0 (8x same program): deg = windowed-reduce of dest-sorted edge weights
#       (ELL layout), dinv = sqrt(1/deg) on device.
#   host: folds dinv[dst]*ew*dinv[src] into selector strip values; builds
#       per-edge x payloads (conv1 needs only 12B/edge of input data, so it
#       streams sequentially -- no gather).
#   L1 (MPMD x8): conv1: stream slot-x payloads + selector strips; per-chunk
#       matmuls reduce into PSUM per 512-dest group; @W1+b1; PE transpose;
#       leaky -> h1 rows (fp16).
#   L2 (MPMD x8): conv2: dma_gather (int16, 6 source windows of 32768, fp16
#       256B rows, 4 SWDGE queues) -> per-chunk matmuls accumulate into
#       memset PSUM; @W2+b2 -> h2T in SBUF; readout MLP on strided graph
#       slices; tanh*90+150 -> y.

import numpy as np

N = 180224
E = 1441792
HID = 128
NPG = 22
NCORES = 8
NLOC = N // NCORES          # 22528 nodes per core
B = N // NPG                # 8192 graphs
BLOC = B // NCORES          # 1024 graphs per core
GROUP = 512                 # dest columns per PSUM bank group
P = 128
WIN = 32768                 # int16 gather window (rows)
NWIN = (N + WIN - 1) // WIN


# ----------------------------------------------------------------------------
# host-side structure building
# ----------------------------------------------------------------------------

def _sorted_edges(srcs, dsts, ews):
    order = np.argsort(dsts, kind="stable")
    return srcs[order].astype(np.int64), dsts[order].astype(np.int64), \
        ews[order].astype(np.float32)


def _build_conv1(ss, ds, es, c):
    """Dest-major whole-run packing into full 128-slot chunks."""
    d_loc = ds - c * NLOC
    deg = np.bincount(d_loc, minlength=NLOC)
    run_start = np.concatenate([[0], np.cumsum(deg)])[:-1]

    n_groups = (NLOC + GROUP - 1) // GROUP
    chunk_group, chunk_lo, chunk_span = [], [], []
    run_chunk = np.empty(NLOC, np.int64)
    run_slot = np.empty(NLOC, np.int64)
    groups = [[] for _ in range(n_groups)]
    acc, cur, cur_g = 0, -1, -1
    deg_l = deg.tolist()
    for dl in range(NLOC):
        g = dl // GROUP
        L = deg_l[dl]
        if cur < 0 or g != cur_g or acc + L > 128:
            cur = len(chunk_group)
            chunk_group.append(g)
            chunk_lo.append(dl)
            chunk_span.append(0)
            groups[g].append(cur)
            acc, cur_g = 0, g
        run_chunk[dl] = cur
        run_slot[dl] = cur * 128 + acc
        acc += L
        chunk_span[cur] = dl - chunk_lo[cur] + 1

    n_chunks = len(chunk_group)
    chunk_lo = np.asarray(chunk_lo, np.int64)
    chunk_span = np.asarray(chunk_span, np.int64)
    sel_off = np.concatenate([[0], np.cumsum(chunk_span)])
    S = int(sel_off[-1])

    rank = np.arange(len(ss)) - run_start[d_loc]
    slot = run_slot[d_loc] + rank
    slots_src = np.zeros(n_chunks * 128, np.int64)
    slots_src[slot] = ss
    ch_of_e = run_chunk[d_loc]
    sel_row = (slot % 128).astype(np.int64)
    sel_col = sel_off[ch_of_e] + d_loc - chunk_lo[ch_of_e]

    wd = int(deg.max())
    ell = np.zeros((NLOC, wd), np.float32)
    ell[d_loc, rank] = es
    return dict(slots_src=slots_src, sel_row=sel_row, sel_col=sel_col, S=S,
                n_chunks=n_chunks, chunk_lo=chunk_lo, chunk_span=chunk_span,
                sel_off=sel_off, groups=groups,
                ell=ell.reshape(P, NLOC // P, wd), wd=wd,
                d_loc=d_loc, src=ss, ew=es)


def _build_conv2(ss, ds, es, c):
    """(group, window, dest)-sorted slots for windowed int16 gathers.

    Per (g, w) run (padded to x32 slots): one dma_gather batch; chunks are
    32-aligned pieces that never cross a slab column; spans accumulate into
    the group's zeroed PSUM bank (start=False everywhere).
    """
    d_loc = ds - c * NLOC
    g_of = d_loc // GROUP
    w_of = ss // WIN
    order = np.lexsort((d_loc, w_of, g_of))
    s2, d2, e2 = ss[order], d_loc[order], es[order]
    n_groups = (NLOC + GROUP - 1) // GROUP

    key = g_of[order] * NWIN + w_of[order]
    bnd = np.flatnonzero(np.diff(key)) + 1
    starts = np.concatenate([[0], bnd])
    ends = np.concatenate([bnd, [len(key)]])

    slot_src = []
    sel_rows, sel_cols, sel_vals = [], [], []
    d_all, s_all = [], []
    batches = []
    sel_off = 0
    idx_cols = 0
    for st, en in zip(starts, ends):
        g = int(key[st] // NWIN)
        w = int(key[st] % NWIN)
        n_raw = en - st
        n_pad = -n_raw % 32
        n = n_raw + n_pad
        srcs_run = np.concatenate([s2[st:en] - w * WIN,
                                   np.zeros(n_pad, np.int64)])
        dls_run = np.concatenate([d2[st:en], np.full(n_pad, d2[en - 1])])
        vals_run = np.concatenate([e2[st:en], np.zeros(n_pad, np.float32)])
        dglob_run = np.concatenate([d2[st:en] + c * NLOC,
                                    np.full(n_pad, d2[en - 1] + c * NLOC)])
        sglob_run = np.concatenate([s2[st:en], np.full(n_pad, w * WIN)])
        chunks = []
        pos = 0
        while pos < n:
            k = min(128 - (pos % 128), n - pos)
            lo = int(dls_run[pos:pos + k].min())
            hi = int(dls_run[pos:pos + k].max())
            span = hi - lo + 1
            chunks.append(dict(col=pos // 128, base=pos % 128, k=int(k),
                               coff=lo - g * GROUP, span=span, soff=sel_off))
            r = np.arange(pos, pos + k)
            sel_rows.append((r % 128).astype(np.int64))
            sel_cols.append(sel_off + dls_run[pos:pos + k] - lo)
            sel_vals.append(vals_run[pos:pos + k])
            d_all.append(dglob_run[pos:pos + k])
            s_all.append(sglob_run[pos:pos + k])
            sel_off += span
            pos += k
        slot_src.append(srcs_run)
        batches.append(dict(g=g, w=w, icol=idx_cols, n=int(n),
                            cols=(n + 127) // 128, chunks=chunks))
        idx_cols += n // 16
    return dict(batches=batches, n_groups=n_groups,
                slot_src=np.concatenate(slot_src),
                sel_row=np.concatenate(sel_rows),
                sel_col=np.concatenate(sel_cols),
                sel_val=np.concatenate(sel_vals),
                d_glob=np.concatenate(d_all),
                s_glob=np.concatenate(s_all),
                S2=int(sel_off), idx_cols=int(idx_cols))


def _conv2_arrays(st, dinv):
    idx = np.zeros((P, st["idx_cols"]), np.int16)
    pos = 0
    for b in st["batches"]:
        n = b["n"]
        blk = st["slot_src"][pos:pos + n].astype(np.int16).reshape(n // 16, 16).T
        idx[:, b["icol"]:b["icol"] + n // 16] = np.tile(blk, (8, 1))
        pos += n
    sel = np.zeros((P, st["S2"]), np.float16)
    vals = st["sel_val"] * dinv[st["d_glob"]] * dinv[st["s_glob"]]
    sel[st["sel_row"], st["sel_col"]] = vals.astype(np.float16)
    return idx, sel


# ----------------------------------------------------------------------------
# device programs
# ----------------------------------------------------------------------------

def _bass_mods():
    import concourse.bass as bass
    import concourse.bacc as bacc
    import concourse.tile as tile
    from concourse import mybir
    return bass, bacc, tile, mybir


def build_l0(wd, nloc=None):
    nloc = NLOC if nloc is None else nloc
    bass, bacc, tile, mybir = _bass_mods()
    nc = bacc.Bacc("TRN2", target_bir_lowering=False, debug=False, num_devices=1)
    cols = nloc // P
    ell = nc.dram_tensor("ell", [P, cols * wd], mybir.dt.float32,
                         kind="ExternalInput").ap()
    dinv = nc.dram_tensor("dinv", [nloc], mybir.dt.float32,
                          kind="ExternalOutput").ap()
    with tile.TileContext(nc) as tc:
        with tc.tile_pool(name="sb", bufs=1) as sb:
            ell_t = sb.tile([P, cols, wd], mybir.dt.float32)
            nc.sync.dma_start(ell_t[:], ell.rearrange("p (c w) -> p c w", w=wd))
            deg_t = sb.tile([P, cols], mybir.dt.float32)
            nc.vector.tensor_reduce(deg_t[:], ell_t[:],
                                    axis=mybir.AxisListType.X,
                                    op=mybir.AluOpType.add)
            rec_t = sb.tile([P, cols], mybir.dt.float32)
            nc.vector.reciprocal(rec_t[:], deg_t[:])
            dv_t = sb.tile([P, cols], mybir.dt.float32)
            nc.scalar.activation(dv_t[:], rec_t[:],
                                 mybir.ActivationFunctionType.Sqrt)
            nc.sync.dma_start(dinv.rearrange("(p c) -> p c", p=P), dv_t[:])
    nc.compile()
    return nc


def build_l1(core, nloc=None):
    """conv1: stream slot-x fp16 payloads + fp16 selector strips; no gather."""
    nloc = NLOC if nloc is None else nloc
    bass, bacc, tile, mybir = _bass_mods()
    from concourse.masks import make_identity
    from contextlib import ExitStack

    n_chunks = core["n_chunks"]
    S = core["S"]
    groups = core["groups"]
    chunk_lo = core["chunk_lo"]
    chunk_span = core["chunk_span"]
    sel_off = core["sel_off"]
    n_groups = len(groups)

    nc = bacc.Bacc("TRN2", target_bir_lowering=False, debug=False, num_devices=1)
    f32 = mybir.dt.float32
    f16 = mybir.dt.float16
    sx = nc.dram_tensor("sx", [P, n_chunks * 3], f16, kind="ExternalInput").ap()
    sel = nc.dram_tensor("sel", [P, S], f16, kind="ExternalInput").ap()
    W1 = nc.dram_tensor("W1", [3, HID], f32, kind="ExternalInput").ap()
    b1 = nc.dram_tensor("b1", [HID, 1], f32, kind="ExternalInput").ap()
    h1 = nc.dram_tensor("h1", [nloc, HID], f16, kind="ExternalOutput").ap()

    max_cg = max(len(g) for g in groups)
    max_selw = int(max(sel_off[g[-1] + 1] - sel_off[g[0]] for g in groups))

    with tile.TileContext(nc) as tc, ExitStack() as ctx:
        consts = ctx.enter_context(tc.tile_pool(name="consts", bufs=1))
        sb = ctx.enter_context(tc.tile_pool(name="sb", bufs=3))
        ps = ctx.enter_context(tc.tile_pool(name="ps", bufs=2, space="PSUM"))
        pst = ctx.enter_context(tc.tile_pool(name="pst", bufs=2, space="PSUM"))

        W1_t = consts.tile([3, HID], f32)
        nc.sync.dma_start(W1_t[:], W1[:])
        b1_t = consts.tile([HID, 1], f32)
        nc.sync.dma_start(b1_t[:], b1[:])
        ident = consts.tile([P, P], f32)
        make_identity(nc, ident)

        for g in range(n_groups):
            chs = groups[g]
            cg = len(chs)
            j0 = chs[0]
            so0 = int(sel_off[j0])
            selw = int(sel_off[chs[-1] + 1]) - so0
            gwidth = min(GROUP, nloc - g * GROUP)

            sx_t = sb.tile([P, max_cg * 3], f16, tag="sx")
            nc.sync.dma_start(sx_t[:, :cg * 3], sx[:, j0 * 3:(j0 + cg) * 3])
            sel_t = sb.tile([P, max_selw], f16, tag="sel")
            nc.sync.dma_start(sel_t[:, :selw], sel[:, so0:so0 + selw])

            agg_ps = ps.tile([3, GROUP], f32, tag="agg")
            for jj, j in enumerate(chs):
                span = int(chunk_span[j])
                coff = int(chunk_lo[j]) - g * GROUP
                soff = int(sel_off[j]) - so0
                nc.tensor.matmul(agg_ps[:, coff:coff + span],
                                 lhsT=sx_t[:, jj * 3:jj * 3 + 3],
                                 rhs=sel_t[:, soff:soff + span],
                                 start=True, stop=True)
            agg_sb = sb.tile([3, GROUP], f32, tag="aggsb")
            nc.vector.tensor_copy(agg_sb[:, :gwidth], agg_ps[:, :gwidth])

            h1T_ps = pst.tile([HID, GROUP], f32, tag="h1T")
            nc.tensor.matmul(h1T_ps[:, :gwidth], lhsT=W1_t[:],
                             rhs=agg_sb[:, :gwidth], start=True, stop=True)
            h1T_sb = sb.tile([HID, GROUP], f32, tag="h1Tsb")
            nc.scalar.activation(h1T_sb[:, :gwidth], h1T_ps[:, :gwidth],
                                 mybir.ActivationFunctionType.Identity,
                                 bias=b1_t[:, 0:1], scale=1.0)

            nt = (gwidth + P - 1) // P
            rows_t = sb.tile([P, nt, HID], f16, tag="rows")
            for tt in range(nt):
                tr_ps = ps.tile([P, P], f32, tag="tr")
                nc.tensor.transpose(tr_ps[:], h1T_sb[:, tt * P:(tt + 1) * P],
                                    ident[:])
                a_t = sb.tile([P, P], f32, tag="lk_a")
                nc.scalar.activation(a_t[:], tr_ps[:],
                                     mybir.ActivationFunctionType.Identity)
                c_t = sb.tile([P, P], f32, tag="lk_b")
                nc.scalar.activation(c_t[:], tr_ps[:],
                                     mybir.ActivationFunctionType.Identity,
                                     scale=0.01)
                nc.vector.tensor_tensor(rows_t[:, tt, :], a_t[:], c_t[:],
                                        op=mybir.AluOpType.max)
            out_ap = h1[g * GROUP:g * GROUP + gwidth, :]
            out_ap = out_ap.rearrange("(t p) f -> p t f", p=P)
            nc.sync.dma_start(out_ap, rows_t[:, :nt, :])
    nc.compile()
    return nc


def build_l2(st, nloc=None, bloc=None, n_rows=None):
    """conv2 (windowed fp16 dma_gather + accumulate) + readout MLP."""
    nloc = NLOC if nloc is None else nloc
    bloc = BLOC if bloc is None else bloc
    n_rows = N if n_rows is None else n_rows
    bass, bacc, tile, mybir = _bass_mods()
    from contextlib import ExitStack

    batches = st["batches"]
    n_groups = st["n_groups"]
    S2 = st["S2"]
    idx_cols = st["idx_cols"]

    nc = bacc.Bacc("TRN2", target_bir_lowering=False, debug=False,
                   num_devices=1, num_swdge_queues=4)
    f32 = mybir.dt.float32
    f16 = mybir.dt.float16
    h1f = nc.dram_tensor("h1f", [n_rows, HID], f16, kind="ExternalInput").ap()
    idx = nc.dram_tensor("idx", [P, idx_cols], mybir.dt.int16,
                         kind="ExternalInput").ap()
    sel = nc.dram_tensor("sel", [P, S2], f16, kind="ExternalInput").ap()
    W2 = nc.dram_tensor("W2", [HID, HID], f32, kind="ExternalInput").ap()
    b2 = nc.dram_tensor("b2", [HID, 1], f32, kind="ExternalInput").ap()
    Wf0 = nc.dram_tensor("Wf0", [HID, NPG * HID], f32, kind="ExternalInput").ap()
    bf0 = nc.dram_tensor("bf0", [HID, 1], f32, kind="ExternalInput").ap()
    Wf1 = nc.dram_tensor("Wf1", [HID, HID], f32, kind="ExternalInput").ap()
    bf1 = nc.dram_tensor("bf1", [HID, 1], f32, kind="ExternalInput").ap()
    Wout = nc.dram_tensor("Wout", [HID, 1], f32, kind="ExternalInput").ap()
    bo = nc.dram_tensor("bo", [1, 1], f32, kind="ExternalInput").ap()
    y = nc.dram_tensor("y", [bloc], f32, kind="ExternalOutput").ap()

    max_cols = max(b["cols"] for b in batches)
    g_first, g_last = {}, {}
    for b in batches:
        ch0, ch1 = b["chunks"][0], b["chunks"][-1]
        g = b["g"]
        if g not in g_first:
            g_first[g] = ch0["soff"]
        g_last[g] = ch1["soff"] + ch1["span"]
    max_gsel = max(g_last[g] - g_first[g] for g in g_first)

    by_group = [[] for _ in range(n_groups)]
    for b in batches:
        by_group[b["g"]].append(b)

    with tile.TileContext(nc) as tc, ExitStack() as ctx:
        consts = ctx.enter_context(tc.tile_pool(name="consts", bufs=1))
        big = ctx.enter_context(tc.tile_pool(name="big", bufs=1))
        sb = ctx.enter_context(tc.tile_pool(name="sb", bufs=2))
        slabs = ctx.enter_context(tc.tile_pool(name="slabs", bufs=6))
        ps = ctx.enter_context(tc.tile_pool(name="ps", bufs=2, space="PSUM"))
        pst = ctx.enter_context(tc.tile_pool(name="pst", bufs=2, space="PSUM"))

        W2_t = consts.tile([HID, HID], f32)
        nc.sync.dma_start(W2_t[:], W2[:])
        b2_t = consts.tile([HID, 1], f32)
        nc.sync.dma_start(b2_t[:], b2[:])
        Wf0_t = consts.tile([HID, NPG, HID], f32)
        nc.sync.dma_start(Wf0_t[:], Wf0.rearrange("k (j m) -> k j m", j=NPG))
        bf0_t = consts.tile([HID, 1], f32)
        nc.sync.dma_start(bf0_t[:], bf0[:])
        Wf1_t = consts.tile([HID, HID], f32)
        nc.sync.dma_start(Wf1_t[:], Wf1[:])
        bf1_t = consts.tile([HID, 1], f32)
        nc.sync.dma_start(bf1_t[:], bf1[:])
        Wout_t = consts.tile([HID, 1], f32)
        nc.sync.dma_start(Wout_t[:], Wout[:])
        bo_t = consts.tile([1, 1], f32)
        nc.sync.dma_start(bo_t[:], bo[:])
        bf0b_t = consts.tile([HID, 1], f32)
        nc.vector.tensor_scalar_mul(bf0b_t[:], bf0_t[:], 0.01)
        bf1b_t = consts.tile([HID, 1], f32)
        nc.vector.tensor_scalar_mul(bf1b_t[:], bf1_t[:], 0.01)

        h2T = big.tile([HID, nloc], f32)
        qn = 0
        for g in range(n_groups):
            gwidth = min(GROUP, nloc - g * GROUP)
            gs0 = g_first[g]
            gselw = g_last[g] - gs0
            sel_t = sb.tile([P, max_gsel], f16, tag="sel")
            nc.sync.dma_start(sel_t[:, :gselw], sel[:, gs0:gs0 + gselw])

            agg_ps = ps.tile([HID, GROUP], f32, tag="agg")
            nc.vector.memset(agg_ps[:], 0.0)
            for b in by_group[g]:
                n, w, cols = b["n"], b["w"], b["cols"]
                wsz = min(WIN, n_rows - w * WIN)
                idx_t = slabs.tile([P, max_cols * 8], mybir.dt.int16, tag="idx")
                nc.sync.dma_start(idx_t[:, :n // 16],
                                  idx[:, b["icol"]:b["icol"] + n // 16])
                gat_t = slabs.tile([P, max_cols, HID], f16, tag="gat")
                nc.gpsimd.dma_gather(
                    out_ap=gat_t[:, :cols, :],
                    in_ap=h1f[w * WIN:w * WIN + wsz, :],
                    idxs_ap=idx_t[:, :n // 16],
                    num_idxs=n, num_idxs_reg=n, elem_size=HID,
                    single_packet=False, queue_num=qn)
                qn = (qn + 1) % 4
                for ch in b["chunks"]:
                    k, base, col = ch["k"], ch["base"], ch["col"]
                    so = ch["soff"] - gs0
                    nc.tensor.matmul(
                        agg_ps[:, ch["coff"]:ch["coff"] + ch["span"]],
                        lhsT=gat_t[base:base + k, col, :],
                        rhs=sel_t[base:base + k, so:so + ch["span"]],
                        start=False, stop=True, skip_group_check=True)

            agg_sb = sb.tile([HID, GROUP], f32, tag="aggsb")
            nc.vector.tensor_copy(agg_sb[:, :gwidth], agg_ps[:, :gwidth])
            h2T_ps = pst.tile([HID, GROUP], f32, tag="h2T")
            nc.tensor.matmul(h2T_ps[:, :gwidth], lhsT=W2_t[:],
                             rhs=agg_sb[:, :gwidth], start=True, stop=True)
            nc.scalar.activation(h2T[:, g * GROUP:g * GROUP + gwidth],
                                 h2T_ps[:, :gwidth],
                                 mybir.ActivationFunctionType.Identity,
                                 bias=b2_t[:, 0:1], scale=1.0)

        # readout MLP, feature-major
        GT = 512
        n_gt = (bloc + GT - 1) // GT
        y_sb = big.tile([1, bloc], f32)
        for gt in range(n_gt):
            gw = min(GT, bloc - gt * GT)
            f0_ps = ps.tile([HID, GT], f32, tag="agg")
            for j in range(NPG):
                zT = h2T[:, gt * GT * NPG + j:
                         gt * GT * NPG + j + (gw - 1) * NPG + 1:NPG]
                nc.tensor.matmul(f0_ps[:, :gw], lhsT=Wf0_t[:, j, :], rhs=zT,
                                 start=(j == 0), stop=(j == NPG - 1))
            a_t = sb.tile([HID, GT], f32, tag="f0a")
            nc.scalar.activation(a_t[:, :gw], f0_ps[:, :gw],
                                 mybir.ActivationFunctionType.Identity,
                                 bias=bf0_t[:, 0:1])
            c_t = sb.tile([HID, GT], f32, tag="f0b")
            nc.scalar.activation(c_t[:, :gw], f0_ps[:, :gw],
                                 mybir.ActivationFunctionType.Identity,
                                 bias=bf0b_t[:, 0:1], scale=0.01)
            f0_t = sb.tile([HID, GT], f32, tag="f0m")
            nc.vector.tensor_tensor(f0_t[:, :gw], a_t[:, :gw], c_t[:, :gw],
                                    op=mybir.AluOpType.max)

            f1_ps = pst.tile([HID, GT], f32, tag="h2T")
            nc.tensor.matmul(f1_ps[:, :gw], lhsT=Wf1_t[:], rhs=f0_t[:, :gw],
                             start=True, stop=True)
            a2_t = sb.tile([HID, GT], f32, tag="f1a")
            nc.scalar.activation(a2_t[:, :gw], f1_ps[:, :gw],
                                 mybir.ActivationFunctionType.Identity,
                                 bias=bf1_t[:, 0:1])
            c2_t = sb.tile([HID, GT], f32, tag="f1b")
            nc.scalar.activation(c2_t[:, :gw], f1_ps[:, :gw],
                                 mybir.ActivationFunctionType.Identity,
                                 bias=bf1b_t[:, 0:1], scale=0.01)
            f1_t = sb.tile([HID, GT], f32, tag="f1m")
            nc.vector.tensor_tensor(f1_t[:, :gw], a2_t[:, :gw], c2_t[:, :gw],
                                    op=mybir.AluOpType.max)

            o_ps = ps.tile([1, GT], f32, tag="o")
            nc.tensor.matmul(o_ps[:, :gw], lhsT=Wout_t[:], rhs=f1_t[:, :gw],
                             start=True, stop=True)
            t_t = sb.tile([1, GT], f32, tag="tanh")
            nc.scalar.activation(t_t[:, :gw], o_ps[:, :gw],
                                 mybir.ActivationFunctionType.Tanh,
                                 bias=bo_t[:, 0:1], scale=1.0)
            nc.vector.tensor_scalar(y_sb[:, gt * GT:gt * GT + gw], t_t[:, :gw],
                                    scalar1=90.0, scalar2=150.0,
                                    op0=mybir.AluOpType.mult,
                                    op1=mybir.AluOpType.add)
        nc.sync.dma_start(y.rearrange("(a b) -> a b", a=1), y_sb[:])
    nc.compile()
    return nc


# ----------------------------------------------------------------------------
# MPMD runner (one program per device, concurrent dispatch)
# ----------------------------------------------------------------------------

def _make_runner(nc, device):
    import jax
    import concourse.mybir as mybir
    from concourse.bass2jax import (install_neuronx_cc_hook, _bass_exec_p,
                                    partition_id_tensor)
    install_neuronx_cc_hook()
    in_names, out_names, out_avals, zero_shapes = [], [], [], []
    part_name = nc.partition_id_tensor.name if nc.partition_id_tensor else None
    for alloc in nc.m.functions[0].allocations:
        if not isinstance(alloc, mybir.MemoryLocationSet):
            continue
        name = alloc.memorylocations[0].name
        if alloc.kind == "ExternalInput":
            if name != part_name:
                in_names.append(name)
        elif alloc.kind == "ExternalOutput":
            out_names.append(name)
            shape = tuple(alloc.tensor_shape)
            dtype = mybir.dt.np(alloc.dtype)
            out_avals.append(jax.core.ShapedArray(shape, dtype))
            zero_shapes.append((shape, dtype))
    n_params = len(in_names)
    all_in = list(in_names) + list(out_names)
    if part_name is not None:
        all_in = all_in + [part_name]
    donate = tuple(range(n_params, n_params + len(out_names)))

    def _body(*args):
        operands = list(args)
        if part_name is not None:
            operands.append(partition_id_tensor())
        outs = _bass_exec_p.bind(
            *operands,
            out_avals=tuple(out_avals),
            in_names=tuple(all_in),
            out_names=tuple(out_names),
            lowering_input_output_aliases=(),
            sim_require_finite=True,
            sim_require_nnan=True,
            nc=nc,
        )
        return tuple(outs)

    jitted = jax.jit(_body, donate_argnums=donate, keep_unused=True)
    return dict(jit=jitted, in_names=in_names, out_names=out_names,
                zero_shapes=zero_shapes, device=device)


def _run_mpmd(runners, in_maps):
    import jax
    from concurrent.futures import ThreadPoolExecutor
    handle_args = []
    for r, m in zip(runners, in_maps):
        args = [jax.device_put(np.ascontiguousarray(m[n]), r["device"])
                for n in r["in_names"]]
        args += [jax.device_put(np.zeros(s, d), r["device"])
                 for s, d in r["zero_shapes"]]
        handle_args.append((r, args))
    with ThreadPoolExecutor(max_workers=max(1, len(runners))) as ex:
        handles = list(ex.map(lambda ra: ra[0]["jit"](*ra[1]), handle_args))
    jax.block_until_ready(handles)
    return [{n: np.asarray(h[i]) for i, n in enumerate(r["out_names"])}
            for r, h in zip(runners, handles)]


BENCH = False
LAST_TIMINGS = {}


def _bench_launch(name, runners, in_maps, iters=3):
    import time as _time
    import jax
    dev_args = []
    for r, m in zip(runners, in_maps):
        dev_args.append([jax.device_put(np.ascontiguousarray(m[n]), r["device"])
                         for n in r["in_names"]])
    best = None
    for _ in range(iters):
        packs = []
        for r, args in zip(runners, dev_args):
            zeros = [jax.device_put(np.zeros(s, d), r["device"])
                     for s, d in r["zero_shapes"]]
            jax.block_until_ready(zeros)
            packs.append((r, args, zeros))
        t0 = _time.perf_counter()
        outs = [r["jit"](*args, *zeros) for r, args, zeros in packs]
        jax.block_until_ready(outs)
        dt = _time.perf_counter() - t0
        best = dt if best is None else min(best, dt)
    LAST_TIMINGS[name] = best


# ----------------------------------------------------------------------------
# top-level kernel
# ----------------------------------------------------------------------------

def kernel(x, edge_index, edge_weight, W1, b1, W2, b2,
           Wf0, bf0, Wf1, bf1, Wout, bout):
    import jax

    x = np.asarray(x, np.float32)
    src = np.asarray(edge_index[0], np.int64)
    dst = np.asarray(edge_index[1], np.int64)
    ew = np.asarray(edge_weight, np.float32)

    loops = np.arange(N, dtype=np.int64)
    srcs = np.concatenate([src, loops])
    dsts = np.concatenate([dst, loops])
    ews = np.concatenate([ew, np.ones(N, np.float32)])
    ss, ds, es = _sorted_edges(srcs, dsts, ews)
    bounds = np.searchsorted(ds, np.arange(NCORES + 1) * NLOC)

    c1, c2 = [], []
    wd = 0
    for c in range(NCORES):
        e0, e1 = bounds[c], bounds[c + 1]
        c1.append(_build_conv1(ss[e0:e1], ds[e0:e1], es[e0:e1], c))
        c2.append(_build_conv2(ss[e0:e1], ds[e0:e1], es[e0:e1], c))
        wd = max(wd, c1[-1]["wd"])

    devices = jax.devices()[:NCORES]

    # ---- L0: degrees -> dinv (device) ----
    nc0 = build_l0(wd)
    l0_runners = [_make_runner(nc0, devices[c]) for c in range(NCORES)]
    l0_ins = []
    for st in c1:
        ell = st["ell"]
        if ell.shape[2] < wd:
            ell = np.concatenate(
                [ell, np.zeros((P, NLOC // P, wd - ell.shape[2]), np.float32)],
                axis=2)
        l0_ins.append({"ell": np.ascontiguousarray(ell).reshape(P, -1)})
    res0 = _run_mpmd(l0_runners, l0_ins)
    dinv = np.concatenate([res0[c]["dinv"] for c in range(NCORES)])
    if BENCH:
        _bench_launch("L0", l0_runners, l0_ins)

    # ---- host: fold normalization into selectors + conv1 payloads ----
    l1_ins = []
    for c, st in enumerate(c1):
        vals = st["ew"] * dinv[st["d_loc"] + c * NLOC] * dinv[st["src"]]
        sel = np.zeros((P, st["S"]), np.float16)
        sel[st["sel_row"], st["sel_col"]] = vals.astype(np.float16)
        sx = x[st["slots_src"]].astype(np.float16)
        sx = np.ascontiguousarray(
            sx.reshape(st["n_chunks"], 128, 3).transpose(1, 0, 2)
        ).reshape(P, st["n_chunks"] * 3)
        l1_ins.append(dict(sx=sx, sel=sel,
                           W1=np.asarray(W1, np.float32),
                           b1=np.asarray(b1, np.float32).reshape(HID, 1)))

    # ---- L1: conv1 ----
    l1_runners = [_make_runner(build_l1(st), devices[c])
                  for c, st in enumerate(c1)]
    res1 = _run_mpmd(l1_runners, l1_ins)
    h1_full = np.concatenate([r["h1"] for r in res1], axis=0)  # fp16
    if BENCH:
        _bench_launch("L1", l1_runners, l1_ins)

    # ---- L2: conv2 + readout ----
    Wf0_r = np.asarray(Wf0, np.float32).reshape(NPG, HID, HID)
    Wf0_r = np.ascontiguousarray(Wf0_r.transpose(1, 0, 2)).reshape(HID, NPG * HID)
    l2_runners = [_make_runner(build_l2(st), devices[c])
                  for c, st in enumerate(c2)]
    l2_ins = []
    for c, st in enumerate(c2):
        idx_arr, sel2 = _conv2_arrays(st, dinv)
        l2_ins.append(dict(h1f=h1_full, idx=idx_arr, sel=sel2,
                           W2=np.asarray(W2, np.float32),
                           b2=np.asarray(b2, np.float32).reshape(HID, 1),
                           Wf0=Wf0_r,
                           bf0=np.asarray(bf0, np.float32).reshape(HID, 1),
                           Wf1=np.asarray(Wf1, np.float32),
                           bf1=np.asarray(bf1, np.float32).reshape(HID, 1),
                           Wout=np.asarray(Wout, np.float32).reshape(HID, 1),
                           bo=np.asarray(bout, np.float32).reshape(1, 1)))
    res2 = _run_mpmd(l2_runners, l2_ins)
    if BENCH:
        _bench_launch("L2", l2_runners, l2_ins)
    y = np.concatenate([r["y"] for r in res2]).reshape(B, 1)
    return y



# revision 19
# speedup vs baseline: 2.1892x; 2.1892x over previous
# Fused single-launch GCN kernel for Trainium2 (8 NeuronCores, SPMD).
#
# Math (PyG GCNConv x2 + per-graph MLP readout):
#   norm[e] = dinv[src]*ew*dinv[dst]  (dinv = rsqrt(weighted indeg + 1))
#   h1 = leaky_relu(scatter(norm*x[src]) + nself*x[d] @ ... W1 + b1)
#   h2 = scatter(norm*h1[src]) @ W2 + b2  (+ self term)
#   y  = MLP(reshape(h2, [B, 22*128]))
#
# Device plan (ONE launch, SPMD over 8 cores, per-core data via inputs):
#   conv1: host-packed slot payloads (x[src], 3 fp16) + selector strips
#     accumulate agg1[3,512] per dest group via PE chunk matmuls; then
#     h1T = Lrelu(W1^T agg1 + b1); pT = W2^T h1T (W2 folded in BEFORE the
#     exchange, by linearity); PE-transpose -> p rows fp16 -> p_local DRAM;
#     self-loop init: agg2 rows = nself[d] * p[d].
#   AllGather p_local (5.8MB/core -> 46MB) on device.
#   conv2: per source-shard window: dma_gather p rows (int16 idx),
#     scale by per-edge norm (DVE broadcast), dma_scatter_add into agg2
#     (fp16 CCE accumulate). No selector matmuls, no W2 matmul after.
#   readout: dma_start_transpose agg2 -> h2T; per-512-graph-tile MLP
#     (b2 folded into bf0 on host); tanh*90+150 -> y [1024]/core.
#
# Structure metadata (chunk counts, window capacities) is computed at
# runtime from the actual edge data but taken as MAX over cores, so the
# single instruction stream is valid for every core (true SPMD).

import numpy as np

N = 180224
E = 1441792
HID = 128
NPG = 22
NCORES = 8
P = 128
GROUP = 512
SPAN1 = 32
VG = GROUP // SPAN1          # cells (32-dest windows) per group
BATCH = 2048                 # conv2 gather batch (slots)
NLOC = N // NCORES           # 22528
BLOC = NLOC // NPG           # 1024 graphs per core
B = N // NPG


# ----------------------------------------------------------------------------
# host-side structure building
# ----------------------------------------------------------------------------

def _prepare(x, src, dst, ew, nloc):
    """Build per-core input arrays + shared (max-over-cores) structure."""
    n = nloc * NCORES
    ncell = (nloc // GROUP) * VG
    deg = np.bincount(dst, weights=ew, minlength=n).astype(np.float64) + 1.0
    dinv = (1.0 / np.sqrt(deg)).astype(np.float32)
    nself = (1.0 / deg).astype(np.float32)
    norm = (dinv[src] * ew * dinv[dst]).astype(np.float32)

    order = np.argsort(dst, kind="stable")
    so, do_, no_ = src[order], dst[order], norm[order]
    bounds = np.searchsorted(do_, np.arange(NCORES + 1) * nloc)

    percore = []
    cnt1 = np.zeros((NCORES, ncell), np.int64)
    cnt2 = np.zeros((NCORES, NCORES), np.int64)
    for c in range(NCORES):
        e0, e1 = bounds[c], bounds[c + 1]
        s2, d2, v2 = so[e0:e1], do_[e0:e1] - c * nloc, no_[e0:e1]
        # conv1 includes self loops as regular slots
        s1 = np.concatenate([s2, np.arange(nloc, dtype=np.int64) + c * nloc])
        d1 = np.concatenate([d2, np.arange(nloc, dtype=np.int64)])
        v1 = np.concatenate([v2, nself[c * nloc:(c + 1) * nloc]])
        cell = d1 // SPAN1
        cnt1[c] = np.bincount(cell, minlength=ncell)
        w2 = s2 // nloc
        cnt2[c] = np.bincount(w2, minlength=NCORES)
        percore.append((s1, d1, v1, cell, s2, d2, v2, w2))

    cap1 = cnt1.max(0)
    chunks1 = (cap1 + P - 1) // P            # >=1 (self loops)
    cbase = np.concatenate([[0], np.cumsum(chunks1)]).astype(np.int64)
    T1 = int(cbase[-1])

    # conv2 rounds: within each source window, slots are split by their
    # occurrence rank per destination, so every scatter-add instruction
    # has UNIQUE destination indices (HW CCE races on duplicates).
    rankmax = 0
    ranks_pc = []
    for c in range(NCORES):
        s2, d2, w2 = percore[c][4], percore[c][5], percore[c][7]
        o2 = np.lexsort((d2, w2))
        d2o, w2o = d2[o2], w2[o2]
        key = w2o * nloc + d2o
        # occurrence rank within (w, dst)
        newrun = np.concatenate([[True], key[1:] != key[:-1]])
        runid = np.cumsum(newrun) - 1
        runstart = np.flatnonzero(newrun)
        rank = np.arange(len(key)) - runstart[runid]
        ranks_pc.append((o2, rank))
        if len(rank):
            rankmax = max(rankmax, int(rank.max()) + 1)
    # counts per (window, round)
    cnt3 = np.zeros((NCORES, NCORES, rankmax), np.int64)
    for c in range(NCORES):
        o2, rank = ranks_pc[c]
        w2o = percore[c][7][o2]
        np.add.at(cnt3[c], (w2o, rank), 1)
    capwr = ((cnt3.max(0) + P - 1) // P * P).astype(np.int64)  # [8, rankmax]
    batches = []
    icols = ncols = 0
    soff = 0
    slotbase = np.zeros((NCORES, rankmax), np.int64)
    for w in range(NCORES):
        for r in range(rankmax):
            if capwr[w, r] == 0:
                continue
            slotbase[w, r] = soff
            off = 0
            while off < capwr[w, r]:
                nb = int(min(BATCH, capwr[w, r] - off))
                batches.append(dict(w=w, nb=nb, io=icols, no=ncols,
                                    so=soff + off))
                icols += nb // 16
                ncols += (nb + P - 1) // P
                off += nb
            soff += int(capwr[w, r])
    TS = soff
    meta = dict(chunks1=chunks1, cbase=cbase, T1=T1, capwr=capwr,
                batches=batches, icols=icols, ncols=ncols, nloc=nloc)

    in_maps = []
    for c in range(NCORES):
        s1, d1, v1, cell, s2, d2, v2, w2 = percore[c]
        o = np.argsort(cell, kind="stable")
        s1o, d1o, v1o, co = s1[o], d1[o], v1[o], cell[o]
        cstart = np.concatenate([[0], np.cumsum(cnt1[c])])
        rank = np.arange(len(co)) - cstart[co]
        slot = cbase[co] * P + rank
        sxf = np.zeros((T1 * P, 3), np.float16)
        sxf[slot] = x[s1o]
        sx = np.ascontiguousarray(
            sxf.reshape(T1, P, 3).transpose(1, 0, 2)).reshape(P, T1 * 3)
        sel = np.zeros((P, T1 * SPAN1), np.float16)
        chunk = cbase[co] + rank // P
        selcol = chunk * SPAN1 + (d1o - co * SPAN1)
        sel[rank % P, selcol] = v1o.astype(np.float16)
        nst = np.ascontiguousarray(
            nself[c * nloc:(c + 1) * nloc].reshape(nloc // P, P).T)

        o2, rank = ranks_pc[c]
        s2o, d2o, v2o, w2o = s2[o2], d2[o2], v2[o2], w2[o2]
        cell2 = w2o * rankmax + rank
        sb_flat = slotbase.reshape(-1)
        p2 = np.argsort(cell2, kind="stable")
        c2s = cell2[p2]
        nr2 = np.concatenate([[True], c2s[1:] != c2s[:-1]])
        rid = np.cumsum(nr2) - 1
        rstart = np.flatnonzero(nr2)
        within = np.arange(len(c2s)) - rstart[rid]
        slot2 = np.empty(len(c2s), np.int64)
        slot2[p2] = sb_flat[c2s] + within
        gidx = np.zeros(TS, np.int16)
        gidx[slot2] = (s2o - w2o * nloc).astype(np.int16)
        sidx = np.full(TS, nloc, np.int16)   # dummy row (norm=0 slots)
        sidx[slot2] = d2o.astype(np.int16)
        nrm = np.zeros(TS, np.float16)
        nrm[slot2] = v2o.astype(np.float16)

        gI = np.zeros((P, icols), np.int16)
        sI = np.zeros((P, icols), np.int16)
        NR = np.zeros((P, ncols), np.float16)
        for b in batches:
            nb, io, no, sof = b["nb"], b["io"], b["no"], b["so"]
            blk = gidx[sof:sof + nb].reshape(nb // 16, 16).T
            gI[:, io:io + nb // 16] = np.tile(blk, (8, 1))
            blk = sidx[sof:sof + nb].reshape(nb // 16, 16).T
            sI[:, io:io + nb // 16] = np.tile(blk, (8, 1))
            cols = (nb + P - 1) // P
            nrb = np.zeros(cols * P, np.float16)
            nrb[:nb] = nrm[sof:sof + nb]
            NR[:, no:no + cols] = nrb.reshape(cols, P).T
        in_maps.append(dict(sx=sx, sel1=sel, nself=nst, gI=gI, sI=sI, nrm=NR))
    return meta, in_maps


def _prep_weights(W1, b1, W2, b2, Wf0, bf0, Wf1, bf1, Wout, bout):
    W1 = np.asarray(W1, np.float32)
    b2 = np.asarray(b2, np.float32).reshape(-1)
    Wf0 = np.asarray(Wf0, np.float32)
    Wf0r = np.ascontiguousarray(
        Wf0.reshape(NPG, HID, HID).transpose(1, 0, 2)).reshape(HID, NPG * HID)
    bf0p = np.asarray(bf0, np.float32).reshape(-1) + np.tile(b2, NPG) @ Wf0
    return dict(
        W1=W1.astype(np.float16),
        b1=np.asarray(b1, np.float32).reshape(HID, 1),
        W2=np.asarray(W2, np.float16),
        Wf0=Wf0r.astype(np.float16),
        bf0=bf0p.astype(np.float32).reshape(HID, 1),
        Wf1=np.asarray(Wf1, np.float16),
        bf1=np.asarray(bf1, np.float32).reshape(HID, 1),
        Wout=np.asarray(Wout, np.float32).astype(np.float16).reshape(HID, 1),
        bo=np.asarray(bout, np.float32).reshape(1, 1),
    )


# ----------------------------------------------------------------------------
# device program
# ----------------------------------------------------------------------------

def _bass_mods():
    import concourse.bass as bass
    import concourse.bacc as bacc
    import concourse.tile as tile
    from concourse import mybir
    return bass, bacc, tile, mybir


def _emit(nc, tc, io, meta, y_ap):
    """Emit the fused program. io: dict name->AP of ExternalInputs."""
    bass, bacc, tile, mybir = _bass_mods()
    from concourse.masks import make_identity
    from contextlib import ExitStack

    f16, f32 = mybir.dt.float16, mybir.dt.float32
    nloc = meta["nloc"]
    ng = nloc // GROUP
    chunks1, cbase, T1 = meta["chunks1"], meta["cbase"], meta["T1"]
    batches = meta["batches"]
    bloc = nloc // NPG
    AF = mybir.ActivationFunctionType

    with ExitStack() as ctx:
        consts = ctx.enter_context(tc.tile_pool(name="consts", bufs=1))
        dram = ctx.enter_context(tc.tile_pool(name="dram", bufs=1, space="DRAM"))

        W1_t = consts.tile([3, HID], f16)
        nc.sync.dma_start(W1_t[:], io["W1"][:])
        b1_t = consts.tile([HID, 1], f32)
        nc.sync.dma_start(b1_t[:], io["b1"][:])
        W2_t = consts.tile([HID, HID], f16)
        nc.sync.dma_start(W2_t[:], io["W2"][:])
        nself_t = consts.tile([P, nloc // P], f32)
        nc.sync.dma_start(nself_t[:], io["nself"][:])
        Wf0_t = consts.tile([HID, NPG, HID], f16)
        nc.sync.dma_start(Wf0_t[:], io["Wf0"].rearrange("k (j m) -> k j m", j=NPG))
        bf0_t = consts.tile([HID, 1], f32)
        nc.sync.dma_start(bf0_t[:], io["bf0"][:])
        Wf1_t = consts.tile([HID, HID], f16)
        nc.sync.dma_start(Wf1_t[:], io["Wf1"][:])
        bf1_t = consts.tile([HID, 1], f32)
        nc.sync.dma_start(bf1_t[:], io["bf1"][:])
        Wout_t = consts.tile([HID, 1], f16)
        nc.sync.dma_start(Wout_t[:], io["Wout"][:])
        bo_t = consts.tile([1, 1], f32)
        nc.sync.dma_start(bo_t[:], io["bo"][:])
        ident = consts.tile([P, P], f16)
        make_identity(nc, ident)
        b1s_t = consts.tile([HID, 1], f32)
        nc.vector.tensor_scalar_mul(b1s_t[:], b1_t[:], 0.01)
        bf0s_t = consts.tile([HID, 1], f32)
        nc.vector.tensor_scalar_mul(bf0s_t[:], bf0_t[:], 0.01)
        bf1s_t = consts.tile([HID, 1], f32)
        nc.vector.tensor_scalar_mul(bf1s_t[:], bf1_t[:], 0.01)

        def lrelu(pool, ps, bias, bias_s, w, tag):
            a_t = pool.tile([HID, w], f32, tag=tag + "a")
            nc.scalar.activation(a_t[:], ps[:], AF.Identity,
                                 bias=bias[:, 0:1])
            c_t = pool.tile([HID, w], f32, tag=tag + "b")
            nc.scalar.activation(c_t[:], ps[:], AF.Identity,
                                 bias=bias_s[:, 0:1], scale=0.01)
            m_t = pool.tile([HID, w], f16, tag=tag + "m")
            nc.vector.tensor_tensor(m_t[:], a_t[:], c_t[:],
                                    op=mybir.AluOpType.max)
            return m_t

        zrow = consts.tile([P, HID], f16)
        nc.vector.memset(zrow[:], 0.0)

        p_loc = dram.tile([nloc, HID], f16)
        p_full = dram.tile([nloc * NCORES, HID], f16)
        agg2 = dram.tile([nloc + P, HID], f16)   # +dummy rows for pad slots

        # ---- conv1 + p = h1@W2 + self-loop init of agg2 ----
        gch = [int(cbase[(g + 1) * VG] - cbase[g * VG]) for g in range(ng)]
        max_gch = max(gch)
        with ExitStack() as c1:
            sb = c1.enter_context(tc.tile_pool(name="sb", bufs=3))
            rows = c1.enter_context(tc.tile_pool(name="rows", bufs=3))
            psA = c1.enter_context(tc.tile_pool(name="psA", bufs=2, space="PSUM"))
            psB = c1.enter_context(tc.tile_pool(name="psB", bufs=2, space="PSUM"))
            psT = c1.enter_context(tc.tile_pool(name="psT", bufs=2, space="PSUM"))
            for g in range(ng):
                q0 = int(cbase[g * VG])
                gc = gch[g]
                sx_t = sb.tile([P, max_gch * 3], f16, tag="sx")
                nc.sync.dma_start(sx_t[:, :gc * 3], io["sx"][:, q0 * 3:(q0 + gc) * 3])
                sl_t = sb.tile([P, max_gch * SPAN1], f16, tag="sel")
                nc.sync.dma_start(sl_t[:, :gc * SPAN1],
                                  io["sel1"][:, q0 * SPAN1:(q0 + gc) * SPAN1])
                agg = psA.tile([3, GROUP], f32, tag="agg")
                for v in range(VG):
                    cell = g * VG + v
                    k = int(chunks1[cell])
                    cq = int(cbase[cell]) - q0
                    for kk in range(k):
                        nc.tensor.matmul(
                            agg[:, v * SPAN1:(v + 1) * SPAN1],
                            lhsT=sx_t[:, (cq + kk) * 3:(cq + kk) * 3 + 3],
                            rhs=sl_t[:, (cq + kk) * SPAN1:(cq + kk + 1) * SPAN1],
                            start=(kk == 0), stop=(kk == k - 1),
                            skip_group_check=True)
                agg_sb = rows.tile([3, GROUP], f16, tag="aggsb")
                nc.vector.tensor_copy(agg_sb[:], agg[:])
                h1_ps = psB.tile([HID, GROUP], f32, tag="mm")
                nc.tensor.matmul(h1_ps[:], lhsT=W1_t[:], rhs=agg_sb[:],
                                 start=True, stop=True)
                h1_sb = lrelu(rows, h1_ps, b1_t, b1s_t, GROUP, "h1")
                p_ps = psB.tile([HID, GROUP], f32, tag="mm")
                nc.tensor.matmul(p_ps[:], lhsT=W2_t[:], rhs=h1_sb[:],
                                 start=True, stop=True)
                p_sb = rows.tile([HID, GROUP], f16, tag="p")
                nc.vector.tensor_copy(p_sb[:], p_ps[:])
                for tt in range(GROUP // P):
                    tr = psT.tile([P, P], f16, tag="tr")
                    nc.tensor.transpose(tr[:], p_sb[:, tt * P:(tt + 1) * P],
                                        ident[:])
                    r_sb = rows.tile([P, P], f16, tag="rows")
                    nc.scalar.activation(r_sb[:], tr[:], AF.Identity)
                    base = g * GROUP + tt * P
                    nc.sync.dma_start(p_loc[base:base + P, :], r_sb[:])
                    s_sb = rows.tile([P, P], f16, tag="self")
                    nc.vector.tensor_scalar_mul(
                        s_sb[:], r_sb[:],
                        nself_t[:, g * (GROUP // P) + tt:g * (GROUP // P) + tt + 1])
                    nc.sync.dma_start(agg2[base:base + P, :], s_sb[:])

        nc.sync.dma_start(agg2[nloc:nloc + P, :], zrow[:])

        # ---- AllGather p ----
        nc.gpsimd.collective_compute(
            "AllGather", mybir.AluOpType.bypass,
            replica_groups=[list(range(NCORES))],
            ins=[p_loc[:, :].opt()], outs=[p_full[:, :].opt()])

        # ---- conv2: gather -> scale -> scatter-add ----
        maxcols = (BATCH + P - 1) // P
        with ExitStack() as c2:
            slabs = c2.enter_context(tc.tile_pool(name="slabs", bufs=3))
            small = c2.enter_context(tc.tile_pool(name="small", bufs=4))
            qn = 0
            for b in batches:
                nb, io_, no, w = b["nb"], b["io"], b["no"], b["w"]
                cols = (nb + P - 1) // P
                gi = small.tile([P, BATCH // 16], mybir.dt.int16, tag="gi")
                nc.sync.dma_start(gi[:, :nb // 16],
                                  io["gI"][:, io_:io_ + nb // 16])
                si = small.tile([P, BATCH // 16], mybir.dt.int16, tag="si")
                nc.sync.dma_start(si[:, :nb // 16],
                                  io["sI"][:, io_:io_ + nb // 16])
                nr = small.tile([P, maxcols, 1], f16, tag="nr")
                nc.sync.dma_start(nr[:, :cols, :],
                                  io["nrm"][:, no:no + cols].rearrange(
                                      "p (c o) -> p c o", o=1))
                gat = slabs.tile([P, maxcols, HID], f16, tag="gat")
                nc.gpsimd.dma_gather(
                    out_ap=gat[:, :cols, :],
                    in_ap=p_full[w * nloc:(w + 1) * nloc, :],
                    idxs_ap=gi[:, :nb // 16],
                    num_idxs=nb, num_idxs_reg=nb, elem_size=HID,
                    single_packet=False, queue_num=qn)
                qs = qn
                g_ap = gat[:, :cols, :]
                n_ap = nr[:, :cols, :]
                g_b, n_b = bass.broadcast_tensor_aps(g_ap, n_ap)
                nc.vector.tensor_tensor(g_ap, g_b, n_b,
                                        op=mybir.AluOpType.mult)
                nc.gpsimd.dma_scatter_add(
                    out_ap=agg2[:, :],
                    in_ap=gat[:, :cols, :],
                    idxs_ap=si[:, :nb // 16],
                    num_idxs=nb, num_idxs_reg=nb, elem_size=HID,
                    queue_num=qs)
                qn = 0

        # ---- readout MLP ----
        GT = min(512, bloc)
        ngt = bloc // GT
        with ExitStack() as c3:
            big = c3.enter_context(tc.tile_pool(name="big", bufs=2))
            ro = c3.enter_context(tc.tile_pool(name="ro", bufs=2))
            rps = c3.enter_context(tc.tile_pool(name="rps", bufs=2, space="PSUM"))
            ops = c3.enter_context(tc.tile_pool(name="ops", bufs=2, space="PSUM"))
            y_sb = consts.tile([1, bloc], f32)
            for gt in range(ngt):
                h2T = big.tile([P, GT * NPG], f16, tag="h2T")
                nc.sync.dma_start_transpose(
                    h2T[:], agg2[gt * GT * NPG:(gt + 1) * GT * NPG, :])
                f0 = rps.tile([HID, GT], f32, tag="f")
                for j in range(NPG):
                    zT = h2T[:, j:j + (GT - 1) * NPG + 1:NPG]
                    nc.tensor.matmul(f0[:], lhsT=Wf0_t[:, j, :], rhs=zT,
                                     start=(j == 0), stop=(j == NPG - 1))
                f0s = lrelu(ro, f0, bf0_t, bf0s_t, GT, "f0")
                f1 = rps.tile([HID, GT], f32, tag="f")
                nc.tensor.matmul(f1[:], lhsT=Wf1_t[:], rhs=f0s[:],
                                 start=True, stop=True)
                f1s = lrelu(ro, f1, bf1_t, bf1s_t, GT, "f1")
                o = ops.tile([1, GT], f32, tag="o")
                nc.tensor.matmul(o[:], lhsT=Wout_t[:], rhs=f1s[:],
                                 start=True, stop=True)
                t = ro.tile([1, GT], f32, tag="t")
                nc.scalar.activation(t[:], o[:], AF.Tanh, bias=bo_t[:, 0:1])
                nc.vector.tensor_scalar(y_sb[:, gt * GT:(gt + 1) * GT], t[:],
                                        scalar1=90.0, scalar2=150.0,
                                        op0=mybir.AluOpType.mult,
                                        op1=mybir.AluOpType.add)
            nc.sync.dma_start(y_ap.rearrange("(a b) -> a b", a=1), y_sb[:])


def build_fused(meta):
    bass, bacc, tile, mybir = _bass_mods()
    f16, f32 = mybir.dt.float16, mybir.dt.float32
    i16 = mybir.dt.int16
    nloc = meta["nloc"]
    bloc = nloc // NPG
    nc = bacc.Bacc("TRN2", target_bir_lowering=False, debug=False,
                   num_devices=NCORES, num_swdge_queues=4)
    io = {}
    T1, icols, ncols = meta["T1"], meta["icols"], meta["ncols"]
    specs = [
        ("sx", [P, T1 * 3], f16), ("sel1", [P, T1 * SPAN1], f16),
        ("nself", [P, nloc // P], f32),
        ("gI", [P, icols], i16), ("sI", [P, icols], i16),
        ("nrm", [P, ncols], f16),
        ("W1", [3, HID], f16), ("b1", [HID, 1], f32),
        ("W2", [HID, HID], f16),
        ("Wf0", [HID, NPG * HID], f16), ("bf0", [HID, 1], f32),
        ("Wf1", [HID, HID], f16), ("bf1", [HID, 1], f32),
        ("Wout", [HID, 1], f16), ("bo", [1, 1], f32),
    ]
    for name, shape, dt in specs:
        io[name] = nc.dram_tensor(name, shape, dt, kind="ExternalInput").ap()
    y = nc.dram_tensor("y", [bloc], f32, kind="ExternalOutput").ap()
    with tile.TileContext(nc) as tc:
        _emit(nc, tc, io, meta, y)
    nc.compile()
    return nc


# ----------------------------------------------------------------------------
# SPMD runner (one program, 8 cores, via PJRT shard_map)
# ----------------------------------------------------------------------------

def _make_runner(nc):
    import jax
    from jax.sharding import Mesh, PartitionSpec
    try:
        from jax.experimental.shard_map import shard_map
    except ImportError:
        from jax.shard_map import shard_map
    import concourse.mybir as mybir
    from concourse.bass2jax import (install_neuronx_cc_hook, _bass_exec_p,
                                    partition_id_tensor)
    install_neuronx_cc_hook()
    part_name = nc.partition_id_tensor.name if nc.partition_id_tensor else None
    in_names, out_names, out_avals, zero_shapes = [], [], [], []
    for alloc in nc.m.functions[0].allocations:
        if not isinstance(alloc, mybir.MemoryLocationSet):
            continue
        name = alloc.memorylocations[0].name
        if alloc.kind == "ExternalInput":
            if name != part_name:
                in_names.append(name)
        elif alloc.kind == "ExternalOutput":
            out_names.append(name)
            shape = tuple(alloc.tensor_shape)
            dtype = mybir.dt.np(alloc.dtype)
            out_avals.append(jax.core.ShapedArray(shape, dtype))
            zero_shapes.append((shape, dtype))
    n_params = len(in_names)
    all_in = list(in_names) + list(out_names)
    if part_name is not None:
        all_in = all_in + [part_name]
    donate = tuple(range(n_params, n_params + len(out_names)))

    def _body(*args):
        operands = list(args)
        if part_name is not None:
            operands.append(partition_id_tensor())
        outs = _bass_exec_p.bind(
            *operands,
            out_avals=tuple(out_avals),
            in_names=tuple(all_in),
            out_names=tuple(out_names),
            lowering_input_output_aliases=(),
            sim_require_finite=True,
            sim_require_nnan=True,
            nc=nc,
        )
        return tuple(outs)

    devices = jax.devices()[:NCORES]
    mesh = Mesh(np.asarray(devices), ("core",))
    in_specs = (PartitionSpec("core"),) * (n_params + len(out_names))
    out_specs = (PartitionSpec("core"),) * len(out_names)
    from jax.experimental.shard_map import shard_map as _sm
    jitted = jax.jit(
        _sm(_body, mesh=mesh, in_specs=in_specs, out_specs=out_specs,
            check_rep=False),
        donate_argnums=donate, keep_unused=True)
    return dict(jit=jitted, in_names=in_names, out_names=out_names,
                zero_shapes=zero_shapes, n_params=n_params,
                out_avals=out_avals)


def _concat_inputs(runner, in_maps):
    cat = []
    for name in runner["in_names"]:
        cat.append(np.concatenate([np.ascontiguousarray(m[name])
                                   for m in in_maps], axis=0))
    return cat


def _run_spmd(runner, in_maps):
    import jax
    cat = _concat_inputs(runner, in_maps)
    zeros = [np.zeros((NCORES * s[0], *s[1:]), d)
             for s, d in runner["zero_shapes"]]
    outs = runner["jit"](*cat, *zeros)
    jax.block_until_ready(outs)
    res = []
    for c in range(NCORES):
        res.append({name: np.asarray(outs[i]).reshape(
            NCORES, *runner["out_avals"][i].shape)[c]
            for i, name in enumerate(runner["out_names"])})
    return res


BENCH = False
LAST_TIMINGS = {}
PIPELINE_TIMINGS = {}


def _bench_launch(name, runner, in_maps, iters=3, pipeline_iters=0):
    import time as _time
    import jax
    cat = [jax.device_put(a) for a in _concat_inputs(runner, in_maps)]
    jax.block_until_ready(cat)
    best = None
    for _ in range(iters):
        zeros = [jax.device_put(np.zeros((NCORES * s[0], *s[1:]), d))
                 for s, d in runner["zero_shapes"]]
        jax.block_until_ready(zeros)
        t0 = _time.perf_counter()
        outs = runner["jit"](*cat, *zeros)
        jax.block_until_ready(outs)
        dt = _time.perf_counter() - t0
        best = dt if best is None else min(best, dt)
    LAST_TIMINGS[name] = best
    if pipeline_iters:
        packs = []
        for _ in range(pipeline_iters):
            zeros = [jax.device_put(np.zeros((NCORES * s[0], *s[1:]), d))
                     for s, d in runner["zero_shapes"]]
            packs.append(zeros)
        jax.block_until_ready(packs)
        t0 = _time.perf_counter()
        outs = [runner["jit"](*cat, *z) for z in packs]
        jax.block_until_ready(outs)
        dt = _time.perf_counter() - t0
        PIPELINE_TIMINGS[name] = dt / pipeline_iters


# ----------------------------------------------------------------------------
# top-level kernel
# ----------------------------------------------------------------------------

def kernel(x, edge_index, edge_weight, W1, b1, W2, b2,
           Wf0, bf0, Wf1, bf1, Wout, bout):
    x = np.asarray(x, np.float32)
    src = np.asarray(edge_index[0], np.int64)
    dst = np.asarray(edge_index[1], np.int64)
    ew = np.asarray(edge_weight, np.float32)

    meta, in_maps = _prepare(x, src, dst, ew, NLOC)
    wts = _prep_weights(W1, b1, W2, b2, Wf0, bf0, Wf1, bf1, Wout, bout)
    for m in in_maps:
        m.update(wts)

    nc = build_fused(meta)
    runner = _make_runner(nc)
    res = _run_spmd(runner, in_maps)
    if BENCH:
        _bench_launch("fused", runner, in_maps, pipeline_iters=8)
    y = np.concatenate([r["y"] for r in res]).reshape(B, 1).astype(np.float32)
    return y


# revision 27
# speedup vs baseline: 2.3829x; 1.0885x over previous
# Fused single-launch GCN kernel for Trainium2 (8 NeuronCores, SPMD).
#
# Math (PyG GCNConv x2 + per-graph MLP readout):
#   norm[e] = dinv[src]*ew*dinv[dst]  (dinv = rsqrt(weighted indeg + 1))
#   h1 = leaky_relu(scatter(norm*x[src]) + nself*x[d] @ ... W1 + b1)
#   h2 = scatter(norm*h1[src]) @ W2 + b2  (+ self term)
#   y  = MLP(reshape(h2, [B, 22*128]))
#
# Device plan (ONE launch, SPMD over 8 cores, per-core data via inputs):
#   conv1: host-packed slot payloads (x[src], 3 fp16) + selector strips
#     accumulate agg1[3,512] per dest group via PE chunk matmuls; then
#     h1T = Lrelu(W1^T agg1 + b1); pT = W2^T h1T (W2 folded in BEFORE the
#     exchange, by linearity); PE-transpose -> p rows fp16 -> p_local DRAM;
#     self-loop init: agg2 rows = nself[d] * p[d].
#   AllGather p_local (5.8MB/core -> 46MB) on device.
#   conv2: per source-shard window: dma_gather p rows (int16 idx),
#     scale by per-edge norm (DVE broadcast), dma_scatter_add into agg2
#     (fp16 CCE accumulate). No selector matmuls, no W2 matmul after.
#   readout: dma_start_transpose agg2 -> h2T; per-512-graph-tile MLP
#     (b2 folded into bf0 on host); tanh*90+150 -> y [1024]/core.
#
# Structure metadata (chunk counts, window capacities) is computed at
# runtime from the actual edge data but taken as MAX over cores, so the
# single instruction stream is valid for every core (true SPMD).

import numpy as np

N = 180224
E = 1441792
HID = 128
NPG = 22
NCORES = 8
P = 128
GROUP = 512
SPAN1 = 8
VG = GROUP // SPAN1          # cells (8-dest windows) per group
BATCH = 2048                 # conv2 gather batch (slots)
NLOC = N // NCORES           # 22528
BLOC = NLOC // NPG           # 1024 graphs per core
B = N // NPG


# ----------------------------------------------------------------------------
# host-side structure building
# ----------------------------------------------------------------------------

def _prepare(x, src, dst, ew, nloc):
    """Build per-core input arrays + shared (max-over-cores) structure."""
    n = nloc * NCORES
    ncell = (nloc // GROUP) * VG
    deg = np.bincount(dst, weights=ew, minlength=n).astype(np.float64) + 1.0
    dinv = (1.0 / np.sqrt(deg)).astype(np.float32)
    nself = (1.0 / deg).astype(np.float32)
    norm = (dinv[src] * ew * dinv[dst]).astype(np.float32)

    order = np.argsort(dst, kind="stable")
    so, do_, no_ = src[order], dst[order], norm[order]
    bounds = np.searchsorted(do_, np.arange(NCORES + 1) * nloc)

    percore = []
    cnt1 = np.zeros((NCORES, ncell), np.int64)
    cnt2 = np.zeros((NCORES, NCORES), np.int64)
    for c in range(NCORES):
        e0, e1 = bounds[c], bounds[c + 1]
        s2, d2, v2 = so[e0:e1], do_[e0:e1] - c * nloc, no_[e0:e1]
        # conv1 includes self loops as regular slots
        s1 = np.concatenate([s2, np.arange(nloc, dtype=np.int64) + c * nloc])
        d1 = np.concatenate([d2, np.arange(nloc, dtype=np.int64)])
        v1 = np.concatenate([v2, nself[c * nloc:(c + 1) * nloc]])
        cell = d1 // SPAN1
        cnt1[c] = np.bincount(cell, minlength=ncell)
        w2 = s2 // nloc
        cnt2[c] = np.bincount(w2, minlength=NCORES)
        percore.append((s1, d1, v1, cell, s2, d2, v2, w2))

    cap1 = cnt1.max(0)
    chunks1 = (cap1 + P - 1) // P            # >=1 (self loops)
    cbase = np.concatenate([[0], np.cumsum(chunks1)]).astype(np.int64)
    T1 = int(cbase[-1])

    # conv2 rounds: within each source window, slots are split by their
    # occurrence rank per destination, so every scatter-add instruction
    # has UNIQUE destination indices (HW CCE races on duplicates).
    rankmax = 0
    ranks_pc = []
    for c in range(NCORES):
        s2, d2, w2 = percore[c][4], percore[c][5], percore[c][7]
        o2 = np.lexsort((d2, w2))
        d2o, w2o = d2[o2], w2[o2]
        key = w2o * nloc + d2o
        # occurrence rank within (w, dst)
        newrun = np.concatenate([[True], key[1:] != key[:-1]])
        runid = np.cumsum(newrun) - 1
        runstart = np.flatnonzero(newrun)
        rank = np.arange(len(key)) - runstart[runid]
        ranks_pc.append((o2, rank))
        if len(rank):
            rankmax = max(rankmax, int(rank.max()) + 1)
    # counts per (window, round)
    cnt3 = np.zeros((NCORES, NCORES, rankmax), np.int64)
    for c in range(NCORES):
        o2, rank = ranks_pc[c]
        w2o = percore[c][7][o2]
        np.add.at(cnt3[c], (w2o, rank), 1)
    capwr = ((cnt3.max(0) + P - 1) // P * P).astype(np.int64)  # [8, rankmax]
    batches = []
    icols = ncols = 0
    soff = 0
    slotbase = np.zeros((NCORES, rankmax), np.int64)
    for w in range(NCORES):
        for r in range(rankmax):
            if capwr[w, r] == 0:
                continue
            slotbase[w, r] = soff
            off = 0
            while off < capwr[w, r]:
                nb = int(min(BATCH, capwr[w, r] - off))
                batches.append(dict(w=w, nb=nb, io=icols, no=ncols,
                                    so=soff + off))
                icols += nb // 16
                ncols += (nb + P - 1) // P
                off += nb
            soff += int(capwr[w, r])
    TS = soff
    meta = dict(chunks1=chunks1, cbase=cbase, T1=T1, capwr=capwr,
                batches=batches, icols=icols, ncols=ncols, nloc=nloc)

    in_maps = []
    for c in range(NCORES):
        s1, d1, v1, cell, s2, d2, v2, w2 = percore[c]
        o = np.argsort(cell, kind="stable")
        s1o, d1o, v1o, co = s1[o], d1[o], v1[o], cell[o]
        cstart = np.concatenate([[0], np.cumsum(cnt1[c])])
        rank = np.arange(len(co)) - cstart[co]
        slot = cbase[co] * P + rank
        sxf = np.zeros((T1 * P, 3), np.float16)
        sxf[slot] = x[s1o]
        sx = np.ascontiguousarray(
            sxf.reshape(T1, P, 3).transpose(1, 0, 2)).reshape(P, T1 * 3)
        sel = np.zeros((P, T1 * SPAN1), np.float16)
        chunk = cbase[co] + rank // P
        selcol = chunk * SPAN1 + (d1o - co * SPAN1)
        sel[rank % P, selcol] = v1o.astype(np.float16)
        nst = np.ascontiguousarray(
            nself[c * nloc:(c + 1) * nloc].reshape(nloc // P, P).T)

        o2, rank = ranks_pc[c]
        s2o, d2o, v2o, w2o = s2[o2], d2[o2], v2[o2], w2[o2]
        cell2 = w2o * rankmax + rank
        sb_flat = slotbase.reshape(-1)
        p2 = np.argsort(cell2, kind="stable")
        c2s = cell2[p2]
        nr2 = np.concatenate([[True], c2s[1:] != c2s[:-1]])
        rid = np.cumsum(nr2) - 1
        rstart = np.flatnonzero(nr2)
        within = np.arange(len(c2s)) - rstart[rid]
        slot2 = np.empty(len(c2s), np.int64)
        slot2[p2] = sb_flat[c2s] + within
        gidx = np.zeros(TS, np.int16)
        gidx[slot2] = (s2o - w2o * nloc).astype(np.int16)
        sidx = np.full(TS, nloc, np.int16)   # dummy row (norm=0 slots)
        sidx[slot2] = d2o.astype(np.int16)
        nrm = np.zeros(TS, np.float16)
        nrm[slot2] = v2o.astype(np.float16)

        gI = np.zeros((16, icols), np.int16)
        sI = np.zeros((16, icols), np.int16)
        NR = np.zeros((P, ncols), np.float16)
        for b in batches:
            nb, io, no, sof = b["nb"], b["io"], b["no"], b["so"]
            gI[:, io:io + nb // 16] = gidx[sof:sof + nb].reshape(nb // 16, 16).T
            sI[:, io:io + nb // 16] = sidx[sof:sof + nb].reshape(nb // 16, 16).T
            cols = (nb + P - 1) // P
            nrb = np.zeros(cols * P, np.float16)
            nrb[:nb] = nrm[sof:sof + nb]
            NR[:, no:no + cols] = nrb.reshape(cols, P).T
        in_maps.append(dict(sx=sx, sel1=sel, nself=nst, gI=gI, sI=sI, nrm=NR))
    return meta, in_maps


def _prep_weights(W1, b1, W2, b2, Wf0, bf0, Wf1, bf1, Wout, bout):
    W1 = np.asarray(W1, np.float32)
    b2 = np.asarray(b2, np.float32).reshape(-1)
    Wf0 = np.asarray(Wf0, np.float32)
    Wf0r = np.ascontiguousarray(
        Wf0.reshape(NPG, HID, HID).transpose(1, 0, 2)).reshape(HID, NPG * HID)
    bf0p = np.asarray(bf0, np.float32).reshape(-1) + np.tile(b2, NPG) @ Wf0
    return dict(
        W1=W1.astype(np.float16),
        b1=np.asarray(b1, np.float32).reshape(HID, 1),
        W2=np.asarray(W2, np.float16),
        Wf0=Wf0r.astype(np.float16),
        bf0=bf0p.astype(np.float32).reshape(HID, 1),
        Wf1=np.asarray(Wf1, np.float16),
        bf1=np.asarray(bf1, np.float32).reshape(HID, 1),
        Wout=np.asarray(Wout, np.float32).astype(np.float16).reshape(HID, 1),
        bo=np.asarray(bout, np.float32).reshape(1, 1),
    )


# ----------------------------------------------------------------------------
# device program
# ----------------------------------------------------------------------------

def _bass_mods():
    import concourse.bass as bass
    import concourse.bacc as bacc
    import concourse.tile as tile
    from concourse import mybir
    return bass, bacc, tile, mybir


def _emit(nc, tc, io, meta, y_ap):
    """Emit the fused program. io: dict name->AP of ExternalInputs."""
    bass, bacc, tile, mybir = _bass_mods()
    from concourse.masks import make_identity
    from contextlib import ExitStack

    f16, f32 = mybir.dt.float16, mybir.dt.float32
    nloc = meta["nloc"]
    ng = nloc // GROUP
    chunks1, cbase, T1 = meta["chunks1"], meta["cbase"], meta["T1"]
    batches = meta["batches"]
    bloc = nloc // NPG
    AF = mybir.ActivationFunctionType

    with ExitStack() as ctx:
        consts = ctx.enter_context(tc.tile_pool(name="consts", bufs=1))
        dram = ctx.enter_context(tc.tile_pool(name="dram", bufs=1, space="DRAM"))

        W1_t = consts.tile([3, HID], f16)
        nc.sync.dma_start(W1_t[:], io["W1"][:])
        b1_t = consts.tile([HID, 1], f32)
        nc.sync.dma_start(b1_t[:], io["b1"][:])
        W2_t = consts.tile([HID, HID], f16)
        nc.sync.dma_start(W2_t[:], io["W2"][:])
        nself_t = consts.tile([P, nloc // P], f32)
        nc.sync.dma_start(nself_t[:], io["nself"][:])
        Wf0_t = consts.tile([HID, NPG, HID], f16)
        nc.sync.dma_start(Wf0_t[:], io["Wf0"].rearrange("k (j m) -> k j m", j=NPG))
        bf0_t = consts.tile([HID, 1], f32)
        nc.sync.dma_start(bf0_t[:], io["bf0"][:])
        Wf1_t = consts.tile([HID, HID], f16)
        nc.sync.dma_start(Wf1_t[:], io["Wf1"][:])
        bf1_t = consts.tile([HID, 1], f32)
        nc.sync.dma_start(bf1_t[:], io["bf1"][:])
        Wout_t = consts.tile([HID, 1], f16)
        nc.sync.dma_start(Wout_t[:], io["Wout"][:])
        bo_t = consts.tile([1, 1], f32)
        nc.sync.dma_start(bo_t[:], io["bo"][:])
        ident = consts.tile([P, P], f16)
        make_identity(nc, ident)
        b1s_t = consts.tile([HID, 1], f32)
        nc.vector.tensor_scalar_mul(b1s_t[:], b1_t[:], 0.01)
        bf0s_t = consts.tile([HID, 1], f32)
        nc.vector.tensor_scalar_mul(bf0s_t[:], bf0_t[:], 0.01)
        bf1s_t = consts.tile([HID, 1], f32)
        nc.vector.tensor_scalar_mul(bf1s_t[:], bf1_t[:], 0.01)

        def lrelu(pool, ps, bias, bias_s, w, tag):
            a_t = pool.tile([HID, w], f32, tag=tag + "a")
            nc.scalar.activation(a_t[:], ps[:], AF.Identity,
                                 bias=bias[:, 0:1])
            c_t = pool.tile([HID, w], f32, tag=tag + "b")
            nc.scalar.activation(c_t[:], ps[:], AF.Identity,
                                 bias=bias_s[:, 0:1], scale=0.01)
            m_t = pool.tile([HID, w], f16, tag=tag + "m")
            nc.vector.tensor_tensor(m_t[:], a_t[:], c_t[:],
                                    op=mybir.AluOpType.max)
            return m_t

        zrow = consts.tile([P, HID], f16)
        nc.vector.memset(zrow[:], 0.0)

        p_loc = dram.tile([nloc, HID], f16)
        p_full = dram.tile([nloc * NCORES, HID], f16)
        agg2 = dram.tile([nloc + P, HID], f16)   # +dummy rows for pad slots

        # ---- conv1 + p = h1@W2 + self-loop init of agg2 ----
        gch = [int(cbase[(g + 1) * VG] - cbase[g * VG]) for g in range(ng)]
        max_gch = max(gch)
        with ExitStack() as c1:
            sb = c1.enter_context(tc.tile_pool(name="sb", bufs=3))
            rows = c1.enter_context(tc.tile_pool(name="rows", bufs=3))
            psA = c1.enter_context(tc.tile_pool(name="psA", bufs=2, space="PSUM"))
            psB = c1.enter_context(tc.tile_pool(name="psB", bufs=2, space="PSUM"))
            psT = c1.enter_context(tc.tile_pool(name="psT", bufs=2, space="PSUM"))
            for g in range(ng):
                q0 = int(cbase[g * VG])
                gc = gch[g]
                sx_t = sb.tile([P, max_gch * 3], f16, tag="sx")
                nc.sync.dma_start(sx_t[:, :gc * 3], io["sx"][:, q0 * 3:(q0 + gc) * 3])
                sl_t = sb.tile([P, max_gch * SPAN1], f16, tag="sel")
                nc.sync.dma_start(sl_t[:, :gc * SPAN1],
                                  io["sel1"][:, q0 * SPAN1:(q0 + gc) * SPAN1])
                agg = psA.tile([3, GROUP], f32, tag="agg")
                for v in range(VG):
                    cell = g * VG + v
                    k = int(chunks1[cell])
                    cq = int(cbase[cell]) - q0
                    for kk in range(k):
                        nc.tensor.matmul(
                            agg[:, v * SPAN1:(v + 1) * SPAN1],
                            lhsT=sx_t[:, (cq + kk) * 3:(cq + kk) * 3 + 3],
                            rhs=sl_t[:, (cq + kk) * SPAN1:(cq + kk + 1) * SPAN1],
                            start=(kk == 0), stop=(kk == k - 1),
                            skip_group_check=True)
                agg_sb = rows.tile([3, GROUP], f16, tag="aggsb")
                nc.vector.tensor_copy(agg_sb[:], agg[:])
                h1_ps = psB.tile([HID, GROUP], f32, tag="mm")
                nc.tensor.matmul(h1_ps[:], lhsT=W1_t[:], rhs=agg_sb[:],
                                 start=True, stop=True)
                h1_sb = lrelu(rows, h1_ps, b1_t, b1s_t, GROUP, "h1")
                p_ps = psB.tile([HID, GROUP], f32, tag="mm")
                nc.tensor.matmul(p_ps[:], lhsT=W2_t[:], rhs=h1_sb[:],
                                 start=True, stop=True)
                p_sb = rows.tile([HID, GROUP], f16, tag="p")
                nc.vector.tensor_copy(p_sb[:], p_ps[:])
                for tt in range(GROUP // P):
                    tr = psT.tile([P, P], f16, tag="tr")
                    nc.tensor.transpose(tr[:], p_sb[:, tt * P:(tt + 1) * P],
                                        ident[:])
                    r_sb = rows.tile([P, P], f16, tag="rows")
                    nc.scalar.activation(r_sb[:], tr[:], AF.Identity)
                    base = g * GROUP + tt * P
                    nc.sync.dma_start(p_loc[base:base + P, :], r_sb[:])
                    s_sb = rows.tile([P, P], f16, tag="self")
                    nc.vector.tensor_scalar_mul(
                        s_sb[:], r_sb[:],
                        nself_t[:, g * (GROUP // P) + tt:g * (GROUP // P) + tt + 1])
                    nc.sync.dma_start(agg2[base:base + P, :], s_sb[:])

        nc.sync.dma_start(agg2[nloc:nloc + P, :], zrow[:])

        # ---- AllGather p ----
        nc.gpsimd.collective_compute(
            "AllGather", mybir.AluOpType.bypass,
            replica_groups=[list(range(NCORES))],
            ins=[p_loc[:, :].opt()], outs=[p_full[:, :].opt()])

        # ---- conv2: gather -> scale -> scatter-add ----
        maxcols = (BATCH + P - 1) // P
        icols = meta["icols"]
        with ExitStack() as c2:
            idxp = c2.enter_context(tc.tile_pool(name="idxp", bufs=1))
            slabs = c2.enter_context(tc.tile_pool(name="slabs", bufs=3))
            small = c2.enter_context(tc.tile_pool(name="small", bufs=4))
            # idx inputs arrive as 16 partitions (2B/slot); replicate the
            # 16-row block to all 128 partitions on-device (ISA reads the
            # idx AP as 8 replicated 16-partition stripes).
            gIt = idxp.tile([P, icols], mybir.dt.int16)
            sIt = idxp.tile([P, icols], mybir.dt.int16)
            nc.sync.dma_start(gIt[0:16, :], io["gI"][:, :])
            nc.sync.dma_start(sIt[0:16, :], io["sI"][:, :])
            for k in range(1, 8):
                nc.sync.dma_start(gIt[16 * k:16 * (k + 1), :], gIt[0:16, :])
                nc.sync.dma_start(sIt[16 * k:16 * (k + 1), :], sIt[0:16, :])
            qn = 0
            for b in batches:
                nb, io_, no, w = b["nb"], b["io"], b["no"], b["w"]
                cols = (nb + P - 1) // P
                gi = gIt[:, io_:io_ + nb // 16]
                si = sIt[:, io_:io_ + nb // 16]
                nr = small.tile([P, maxcols, 1], f16, tag="nr")
                nc.sync.dma_start(nr[:, :cols, :],
                                  io["nrm"][:, no:no + cols].rearrange(
                                      "p (c o) -> p c o", o=1))
                gat = slabs.tile([P, maxcols, HID], f16, tag="gat")
                nc.gpsimd.dma_gather(
                    out_ap=gat[:, :cols, :],
                    in_ap=p_full[w * nloc:(w + 1) * nloc, :],
                    idxs_ap=gi,
                    num_idxs=nb, num_idxs_reg=nb, elem_size=HID,
                    single_packet=False, queue_num=qn)
                qs = qn
                g_ap = gat[:, :cols, :]
                n_ap = nr[:, :cols, :]
                g_b, n_b = bass.broadcast_tensor_aps(g_ap, n_ap)
                nc.vector.tensor_tensor(g_ap, g_b, n_b,
                                        op=mybir.AluOpType.mult)
                nc.gpsimd.dma_scatter_add(
                    out_ap=agg2[:, :],
                    in_ap=gat[:, :cols, :],
                    idxs_ap=si,
                    num_idxs=nb, num_idxs_reg=nb, elem_size=HID,
                    queue_num=qs)
                qn = 0

        # ---- readout MLP ----
        GT = min(512, bloc)
        ngt = bloc // GT
        with ExitStack() as c3:
            big = c3.enter_context(tc.tile_pool(name="big", bufs=2))
            ro = c3.enter_context(tc.tile_pool(name="ro", bufs=2))
            rps = c3.enter_context(tc.tile_pool(name="rps", bufs=2, space="PSUM"))
            ops = c3.enter_context(tc.tile_pool(name="ops", bufs=2, space="PSUM"))
            y_sb = consts.tile([1, bloc], f32)
            for gt in range(ngt):
                h2T = big.tile([P, GT * NPG], f16, tag="h2T")
                nc.sync.dma_start_transpose(
                    h2T[:], agg2[gt * GT * NPG:(gt + 1) * GT * NPG, :])
                f0 = rps.tile([HID, GT], f32, tag="f")
                for j in range(NPG):
                    zT = h2T[:, j:j + (GT - 1) * NPG + 1:NPG]
                    nc.tensor.matmul(f0[:], lhsT=Wf0_t[:, j, :], rhs=zT,
                                     start=(j == 0), stop=(j == NPG - 1))
                f0s = lrelu(ro, f0, bf0_t, bf0s_t, GT, "f0")
                f1 = rps.tile([HID, GT], f32, tag="f")
                nc.tensor.matmul(f1[:], lhsT=Wf1_t[:], rhs=f0s[:],
                                 start=True, stop=True)
                f1s = lrelu(ro, f1, bf1_t, bf1s_t, GT, "f1")
                o = ops.tile([1, GT], f32, tag="o")
                nc.tensor.matmul(o[:], lhsT=Wout_t[:], rhs=f1s[:],
                                 start=True, stop=True)
                t = ro.tile([1, GT], f32, tag="t")
                nc.scalar.activation(t[:], o[:], AF.Tanh, bias=bo_t[:, 0:1])
                nc.vector.tensor_scalar(y_sb[:, gt * GT:(gt + 1) * GT], t[:],
                                        scalar1=90.0, scalar2=150.0,
                                        op0=mybir.AluOpType.mult,
                                        op1=mybir.AluOpType.add)
            nc.sync.dma_start(y_ap.rearrange("(a b) -> a b", a=1), y_sb[:])


def build_fused(meta):
    bass, bacc, tile, mybir = _bass_mods()
    f16, f32 = mybir.dt.float16, mybir.dt.float32
    i16 = mybir.dt.int16
    nloc = meta["nloc"]
    bloc = nloc // NPG
    nc = bacc.Bacc("TRN2", target_bir_lowering=False, debug=False,
                   num_devices=NCORES, num_swdge_queues=4)
    io = {}
    T1, icols, ncols = meta["T1"], meta["icols"], meta["ncols"]
    specs = [
        ("sx", [P, T1 * 3], f16), ("sel1", [P, T1 * SPAN1], f16),
        ("nself", [P, nloc // P], f32),
        ("gI", [16, icols], i16), ("sI", [16, icols], i16),
        ("nrm", [P, ncols], f16),
        ("W1", [3, HID], f16), ("b1", [HID, 1], f32),
        ("W2", [HID, HID], f16),
        ("Wf0", [HID, NPG * HID], f16), ("bf0", [HID, 1], f32),
        ("Wf1", [HID, HID], f16), ("bf1", [HID, 1], f32),
        ("Wout", [HID, 1], f16), ("bo", [1, 1], f32),
    ]
    for name, shape, dt in specs:
        io[name] = nc.dram_tensor(name, shape, dt, kind="ExternalInput").ap()
    y = nc.dram_tensor("y", [bloc], f32, kind="ExternalOutput").ap()
    with tile.TileContext(nc) as tc:
        _emit(nc, tc, io, meta, y)
    nc.compile()
    return nc


# ----------------------------------------------------------------------------
# SPMD runner (one program, 8 cores, via PJRT shard_map)
# ----------------------------------------------------------------------------

def _make_runner(nc):
    import jax
    from jax.sharding import Mesh, PartitionSpec
    try:
        from jax.experimental.shard_map import shard_map
    except ImportError:
        from jax.shard_map import shard_map
    import concourse.mybir as mybir
    from concourse.bass2jax import (install_neuronx_cc_hook, _bass_exec_p,
                                    partition_id_tensor)
    install_neuronx_cc_hook()
    part_name = nc.partition_id_tensor.name if nc.partition_id_tensor else None
    in_names, out_names, out_avals, zero_shapes = [], [], [], []
    for alloc in nc.m.functions[0].allocations:
        if not isinstance(alloc, mybir.MemoryLocationSet):
            continue
        name = alloc.memorylocations[0].name
        if alloc.kind == "ExternalInput":
            if name != part_name:
                in_names.append(name)
        elif alloc.kind == "ExternalOutput":
            out_names.append(name)
            shape = tuple(alloc.tensor_shape)
            dtype = mybir.dt.np(alloc.dtype)
            out_avals.append(jax.core.ShapedArray(shape, dtype))
            zero_shapes.append((shape, dtype))
    n_params = len(in_names)
    all_in = list(in_names) + list(out_names)
    if part_name is not None:
        all_in = all_in + [part_name]
    donate = tuple(range(n_params, n_params + len(out_names)))

    def _body(*args):
        operands = list(args)
        if part_name is not None:
            operands.append(partition_id_tensor())
        outs = _bass_exec_p.bind(
            *operands,
            out_avals=tuple(out_avals),
            in_names=tuple(all_in),
            out_names=tuple(out_names),
            lowering_input_output_aliases=(),
            sim_require_finite=True,
            sim_require_nnan=True,
            nc=nc,
        )
        return tuple(outs)

    devices = jax.devices()[:NCORES]
    mesh = Mesh(np.asarray(devices), ("core",))
    in_specs = (PartitionSpec("core"),) * (n_params + len(out_names))
    out_specs = (PartitionSpec("core"),) * len(out_names)
    from jax.experimental.shard_map import shard_map as _sm
    jitted = jax.jit(
        _sm(_body, mesh=mesh, in_specs=in_specs, out_specs=out_specs,
            check_rep=False),
        donate_argnums=donate, keep_unused=True)
    return dict(jit=jitted, in_names=in_names, out_names=out_names,
                zero_shapes=zero_shapes, n_params=n_params,
                out_avals=out_avals)


def _concat_inputs(runner, in_maps):
    cat = []
    for name in runner["in_names"]:
        cat.append(np.concatenate([np.ascontiguousarray(m[name])
                                   for m in in_maps], axis=0))
    return cat


def _run_spmd(runner, in_maps):
    import jax
    cat = _concat_inputs(runner, in_maps)
    zeros = [np.zeros((NCORES * s[0], *s[1:]), d)
             for s, d in runner["zero_shapes"]]
    outs = runner["jit"](*cat, *zeros)
    jax.block_until_ready(outs)
    res = []
    for c in range(NCORES):
        res.append({name: np.asarray(outs[i]).reshape(
            NCORES, *runner["out_avals"][i].shape)[c]
            for i, name in enumerate(runner["out_names"])})
    return res


BENCH = False
LAST_TIMINGS = {}
PIPELINE_TIMINGS = {}


def _bench_launch(name, runner, in_maps, iters=3, pipeline_iters=0):
    import time as _time
    import jax
    cat = [jax.device_put(a) for a in _concat_inputs(runner, in_maps)]
    jax.block_until_ready(cat)
    best = None
    for _ in range(iters):
        zeros = [jax.device_put(np.zeros((NCORES * s[0], *s[1:]), d))
                 for s, d in runner["zero_shapes"]]
        jax.block_until_ready(zeros)
        t0 = _time.perf_counter()
        outs = runner["jit"](*cat, *zeros)
        jax.block_until_ready(outs)
        dt = _time.perf_counter() - t0
        best = dt if best is None else min(best, dt)
    LAST_TIMINGS[name] = best
    if pipeline_iters:
        packs = []
        for _ in range(pipeline_iters):
            zeros = [jax.device_put(np.zeros((NCORES * s[0], *s[1:]), d))
                     for s, d in runner["zero_shapes"]]
            packs.append(zeros)
        jax.block_until_ready(packs)
        t0 = _time.perf_counter()
        outs = [runner["jit"](*cat, *z) for z in packs]
        jax.block_until_ready(outs)
        dt = _time.perf_counter() - t0
        PIPELINE_TIMINGS[name] = dt / pipeline_iters


# ----------------------------------------------------------------------------
# top-level kernel
# ----------------------------------------------------------------------------

def kernel(x, edge_index, edge_weight, W1, b1, W2, b2,
           Wf0, bf0, Wf1, bf1, Wout, bout):
    x = np.asarray(x, np.float32)
    src = np.asarray(edge_index[0], np.int64)
    dst = np.asarray(edge_index[1], np.int64)
    ew = np.asarray(edge_weight, np.float32)

    meta, in_maps = _prepare(x, src, dst, ew, NLOC)
    wts = _prep_weights(W1, b1, W2, b2, Wf0, bf0, Wf1, bf1, Wout, bout)
    for m in in_maps:
        m.update(wts)

    nc = build_fused(meta)
    runner = _make_runner(nc)
    res = _run_spmd(runner, in_maps)
    if BENCH:
        _bench_launch("fused", runner, in_maps, pipeline_iters=8)
    y = np.concatenate([r["y"] for r in res]).reshape(B, 1).astype(np.float32)
    return y


# revision 34
# speedup vs baseline: 2.6095x; 1.0951x over previous
# Fused single-launch GCN kernel for Trainium2 (8 NeuronCores, SPMD).
#
# Math (PyG GCNConv x2 + per-graph MLP readout):
#   norm[e] = dinv[src]*ew*dinv[dst]  (dinv = rsqrt(weighted indeg + 1))
#   h1 = leaky_relu(scatter(norm*x[src]) + nself*x[d] @ ... W1 + b1)
#   h2 = scatter(norm*h1[src]) @ W2 + b2  (+ self term)
#   y  = MLP(reshape(h2, [B, 22*128]))
#
# Device plan (ONE launch, SPMD over 8 cores, per-core data via inputs):
#   conv1: host-packed slot payloads (x[src], 3 fp16) + selector strips
#     accumulate agg1[3,512] per dest group via PE chunk matmuls; then
#     h1T = Lrelu(W1^T agg1 + b1); pT = W2^T h1T (W2 folded in BEFORE the
#     exchange, by linearity); PE-transpose -> p rows fp16 -> p_local DRAM;
#     self-loop init: agg2 rows = nself[d] * p[d].
#   AllGather p_local (5.8MB/core -> 46MB) on device.
#   conv2: per source-shard window: dma_gather p rows (int16 idx),
#     scale by per-edge norm (DVE broadcast), dma_scatter_add into agg2
#     (fp16 CCE accumulate). No selector matmuls, no W2 matmul after.
#   readout: dma_start_transpose agg2 -> h2T; per-512-graph-tile MLP
#     (b2 folded into bf0 on host); tanh*90+150 -> y [1024]/core.
#
# Structure metadata (chunk counts, window capacities) is computed at
# runtime from the actual edge data but taken as MAX over cores, so the
# single instruction stream is valid for every core (true SPMD).

import numpy as np

N = 180224
E = 1441792
HID = 128
NPG = 22
NCORES = 8
P = 128
GROUP = 512
SPAN1 = 8
VG = GROUP // SPAN1          # cells (8-dest windows) per group
BATCH = 2048                 # conv2 gather batch (slots)
NLOC = N // NCORES           # 22528
BLOC = NLOC // NPG           # 1024 graphs per core
B = N // NPG


# ----------------------------------------------------------------------------
# host-side structure building
# ----------------------------------------------------------------------------

def _prepare(x, src, dst, ew, nloc):
    """Build per-core input arrays + shared (max-over-cores) structure."""
    n = nloc * NCORES
    ncell = (nloc // GROUP) * VG
    deg = np.bincount(dst, weights=ew, minlength=n).astype(np.float64) + 1.0
    dinv = (1.0 / np.sqrt(deg)).astype(np.float32)
    nself = (1.0 / deg).astype(np.float32)
    norm = (dinv[src] * ew * dinv[dst]).astype(np.float32)

    order = np.argsort(dst, kind="stable")
    so, do_, no_ = src[order], dst[order], norm[order]
    bounds = np.searchsorted(do_, np.arange(NCORES + 1) * nloc)

    percore = []
    cnt1 = np.zeros((NCORES, ncell), np.int64)
    cnt2 = np.zeros((NCORES, NCORES), np.int64)
    for c in range(NCORES):
        e0, e1 = bounds[c], bounds[c + 1]
        s2, d2, v2 = so[e0:e1], do_[e0:e1] - c * nloc, no_[e0:e1]
        # conv1 includes self loops as regular slots
        s1 = np.concatenate([s2, np.arange(nloc, dtype=np.int64) + c * nloc])
        d1 = np.concatenate([d2, np.arange(nloc, dtype=np.int64)])
        v1 = np.concatenate([v2, nself[c * nloc:(c + 1) * nloc]])
        cell = d1 // SPAN1
        cnt1[c] = np.bincount(cell, minlength=ncell)
        w2 = s2 // nloc
        cnt2[c] = np.bincount(w2, minlength=NCORES)
        percore.append((s1, d1, v1, cell, s2, d2, v2, w2))

    cap1 = cnt1.max(0)
    chunks1 = (cap1 + P - 1) // P            # >=1 (self loops)
    cbase = np.concatenate([[0], np.cumsum(chunks1)]).astype(np.int64)
    T1 = int(cbase[-1])

    # conv2 rounds: within each source window, slots are split by their
    # occurrence rank per destination, so every scatter-add instruction
    # has UNIQUE destination indices (HW CCE races on duplicates).
    rankmax = 0
    ranks_pc = []
    for c in range(NCORES):
        s2, d2, w2 = percore[c][4], percore[c][5], percore[c][7]
        o2 = np.lexsort((d2, w2))
        d2o, w2o = d2[o2], w2[o2]
        key = w2o * nloc + d2o
        # occurrence rank within (w, dst)
        newrun = np.concatenate([[True], key[1:] != key[:-1]])
        runid = np.cumsum(newrun) - 1
        runstart = np.flatnonzero(newrun)
        rank = np.arange(len(key)) - runstart[runid]
        ranks_pc.append((o2, rank))
        if len(rank):
            rankmax = max(rankmax, int(rank.max()) + 1)
    # counts per (window, round)
    cnt3 = np.zeros((NCORES, NCORES, rankmax), np.int64)
    for c in range(NCORES):
        o2, rank = ranks_pc[c]
        w2o = percore[c][7][o2]
        np.add.at(cnt3[c], (w2o, rank), 1)
    capwr = ((cnt3.max(0) + P - 1) // P * P).astype(np.int64)  # [8, rankmax]
    batches = []
    icols = ncols = 0
    soff = 0
    slotbase = np.zeros((NCORES, rankmax), np.int64)
    for w in range(NCORES):
        for r in range(rankmax):
            if capwr[w, r] == 0:
                continue
            slotbase[w, r] = soff
            off = 0
            while off < capwr[w, r]:
                nb = int(min(BATCH, capwr[w, r] - off))
                batches.append(dict(w=w, nb=nb, io=icols, no=ncols,
                                    so=soff + off))
                icols += nb // 16
                ncols += (nb + P - 1) // P
                off += nb
            soff += int(capwr[w, r])
    TS = soff
    meta = dict(chunks1=chunks1, cbase=cbase, T1=T1, capwr=capwr,
                batches=batches, icols=icols, ncols=ncols, nloc=nloc)

    in_maps = []
    for c in range(NCORES):
        s1, d1, v1, cell, s2, d2, v2, w2 = percore[c]
        o = np.argsort(cell, kind="stable")
        s1o, d1o, v1o, co = s1[o], d1[o], v1[o], cell[o]
        cstart = np.concatenate([[0], np.cumsum(cnt1[c])])
        rank = np.arange(len(co)) - cstart[co]
        slot = cbase[co] * P + rank
        import ml_dtypes
        f8 = ml_dtypes.float8_e4m3
        sxf = np.zeros((T1 * P, 3), f8)
        sxf[slot] = x[s1o].astype(f8)
        sx = np.ascontiguousarray(
            sxf.reshape(T1, P, 3).transpose(1, 0, 2)).reshape(P, T1 * 3)
        sel = np.zeros((P, T1 * SPAN1), f8)
        chunk = cbase[co] + rank // P
        selcol = chunk * SPAN1 + (d1o - co * SPAN1)
        sel[rank % P, selcol] = v1o.astype(f8)
        # ship fp8 bytes as int8 (XLA on TRN2 rejects fp8 dtypes)
        sx = sx.view(np.int8)
        sel = sel.view(np.int8)
        nst = np.ascontiguousarray(
            nself[c * nloc:(c + 1) * nloc].reshape(nloc // P, P).T)

        o2, rank = ranks_pc[c]
        s2o, d2o, v2o, w2o = s2[o2], d2[o2], v2[o2], w2[o2]
        cell2 = w2o * rankmax + rank
        sb_flat = slotbase.reshape(-1)
        p2 = np.argsort(cell2, kind="stable")
        c2s = cell2[p2]
        nr2 = np.concatenate([[True], c2s[1:] != c2s[:-1]])
        rid = np.cumsum(nr2) - 1
        rstart = np.flatnonzero(nr2)
        within = np.arange(len(c2s)) - rstart[rid]
        slot2 = np.empty(len(c2s), np.int64)
        slot2[p2] = sb_flat[c2s] + within
        gidx = np.zeros(TS, np.int16)
        gidx[slot2] = (s2o - w2o * nloc).astype(np.int16)
        sidx = np.full(TS, nloc, np.int16)   # dummy row (norm=0 slots)
        sidx[slot2] = d2o.astype(np.int16)
        nrm = np.zeros(TS, np.float16)
        nrm[slot2] = v2o.astype(np.float16)

        gI = np.zeros((16, icols), np.int16)
        sI = np.zeros((16, icols), np.int16)
        NR = np.zeros((P, ncols), np.float16)
        for b in batches:
            nb, io, no, sof = b["nb"], b["io"], b["no"], b["so"]
            gI[:, io:io + nb // 16] = gidx[sof:sof + nb].reshape(nb // 16, 16).T
            sI[:, io:io + nb // 16] = sidx[sof:sof + nb].reshape(nb // 16, 16).T
            cols = (nb + P - 1) // P
            nrb = np.zeros(cols * P, np.float16)
            nrb[:nb] = nrm[sof:sof + nb]
            NR[:, no:no + cols] = nrb.reshape(cols, P).T
        in_maps.append(dict(sx=sx, sel1=sel, nself=nst, gI=gI, sI=sI, nrm=NR))
    return meta, in_maps


def _prep_weights(W1, b1, W2, b2, Wf0, bf0, Wf1, bf1, Wout, bout):
    W1 = np.asarray(W1, np.float32)
    b2 = np.asarray(b2, np.float32).reshape(-1)
    Wf0 = np.asarray(Wf0, np.float32)
    Wf0r = np.ascontiguousarray(
        Wf0.reshape(NPG, HID, HID).transpose(1, 0, 2)).reshape(HID, NPG * HID)
    bf0p = np.asarray(bf0, np.float32).reshape(-1) + np.tile(b2, NPG) @ Wf0
    return dict(
        W1=W1.astype(np.float16),
        b1=np.asarray(b1, np.float32).reshape(HID, 1),
        W2=np.asarray(W2, np.float16),
        Wf0=Wf0r.astype(np.float16),
        bf0=bf0p.astype(np.float32).reshape(HID, 1),
        Wf1=np.asarray(Wf1, np.float16),
        bf1=np.asarray(bf1, np.float32).reshape(HID, 1),
        Wout=np.asarray(Wout, np.float32).astype(np.float16).reshape(HID, 1),
        bo=np.asarray(bout, np.float32).reshape(1, 1),
    )


# ----------------------------------------------------------------------------
# device program
# ----------------------------------------------------------------------------

def _bass_mods():
    import concourse.bass as bass
    import concourse.bacc as bacc
    import concourse.tile as tile
    from concourse import mybir
    return bass, bacc, tile, mybir


def _emit(nc, tc, io, meta, y_ap):
    """Emit the fused program. io: dict name->AP of ExternalInputs."""
    bass, bacc, tile, mybir = _bass_mods()
    from concourse.masks import make_identity
    from contextlib import ExitStack

    f16, f32 = mybir.dt.float16, mybir.dt.float32
    nloc = meta["nloc"]
    ng = nloc // GROUP
    chunks1, cbase, T1 = meta["chunks1"], meta["cbase"], meta["T1"]
    batches = meta["batches"]
    bloc = nloc // NPG
    AF = mybir.ActivationFunctionType

    with ExitStack() as ctx:
        consts = ctx.enter_context(tc.tile_pool(name="consts", bufs=1))
        dram = ctx.enter_context(tc.tile_pool(name="dram", bufs=1, space="DRAM"))

        W1_t = consts.tile([3, HID], f16)
        nc.sync.dma_start(W1_t[:], io["W1"][:])
        b1_t = consts.tile([HID, 1], f32)
        nc.sync.dma_start(b1_t[:], io["b1"][:])
        W2_t = consts.tile([HID, HID], f16)
        nc.sync.dma_start(W2_t[:], io["W2"][:])
        nself_t = consts.tile([P, nloc // P], f32)
        nc.sync.dma_start(nself_t[:], io["nself"][:])
        Wf0_t = consts.tile([HID, NPG, HID], f16)
        nc.sync.dma_start(Wf0_t[:], io["Wf0"].rearrange("k (j m) -> k j m", j=NPG))
        bf0_t = consts.tile([HID, 1], f32)
        nc.sync.dma_start(bf0_t[:], io["bf0"][:])
        Wf1_t = consts.tile([HID, HID], f16)
        nc.sync.dma_start(Wf1_t[:], io["Wf1"][:])
        bf1_t = consts.tile([HID, 1], f32)
        nc.sync.dma_start(bf1_t[:], io["bf1"][:])
        Wout_t = consts.tile([HID, 1], f16)
        nc.sync.dma_start(Wout_t[:], io["Wout"][:])
        bo_t = consts.tile([1, 1], f32)
        nc.sync.dma_start(bo_t[:], io["bo"][:])
        ident = consts.tile([P, P], f16)
        make_identity(nc, ident)
        b1s_t = consts.tile([HID, 1], f32)
        nc.vector.tensor_scalar_mul(b1s_t[:], b1_t[:], 0.01)
        bf0s_t = consts.tile([HID, 1], f32)
        nc.vector.tensor_scalar_mul(bf0s_t[:], bf0_t[:], 0.01)
        bf1s_t = consts.tile([HID, 1], f32)
        nc.vector.tensor_scalar_mul(bf1s_t[:], bf1_t[:], 0.01)

        def lrelu(pool, ps, bias, bias_s, w, tag):
            a_t = pool.tile([HID, w], f32, tag=tag + "a")
            nc.scalar.activation(a_t[:], ps[:], AF.Identity,
                                 bias=bias[:, 0:1])
            c_t = pool.tile([HID, w], f32, tag=tag + "b")
            nc.scalar.activation(c_t[:], ps[:], AF.Identity,
                                 bias=bias_s[:, 0:1], scale=0.01)
            m_t = pool.tile([HID, w], f16, tag=tag + "m")
            nc.vector.tensor_tensor(m_t[:], a_t[:], c_t[:],
                                    op=mybir.AluOpType.max)
            return m_t

        zrow = consts.tile([P, HID], f16)
        nc.vector.memset(zrow[:], 0.0)

        p_loc = dram.tile([nloc, HID], f16)
        p_full = dram.tile([nloc * NCORES, HID], f16)
        agg2 = dram.tile([nloc + P, HID], f16)   # +dummy rows for pad slots

        # ---- conv1 + p = h1@W2 + self-loop init of agg2 ----
        gch = [int(cbase[(g + 1) * VG] - cbase[g * VG]) for g in range(ng)]
        max_gch = max(gch)
        with ExitStack() as c1:
            sb = c1.enter_context(tc.tile_pool(name="sb", bufs=3))
            rows = c1.enter_context(tc.tile_pool(name="rows", bufs=3))
            psA = c1.enter_context(tc.tile_pool(name="psA", bufs=2, space="PSUM"))
            psB = c1.enter_context(tc.tile_pool(name="psB", bufs=2, space="PSUM"))
            psT = c1.enter_context(tc.tile_pool(name="psT", bufs=2, space="PSUM"))
            for g in range(ng):
                q0 = int(cbase[g * VG])
                gc = gch[g]
                f8 = mybir.dt.float8e4
                sx_t = sb.tile([P, max_gch * 3], f8, tag="sx")
                nc.sync.dma_start(sx_t[:, :gc * 3],
                                  io["sx"][:, q0 * 3:(q0 + gc) * 3].bitcast(f8))
                sl_t = sb.tile([P, max_gch * SPAN1], f8, tag="sel")
                nc.sync.dma_start(
                    sl_t[:, :gc * SPAN1],
                    io["sel1"][:, q0 * SPAN1:(q0 + gc) * SPAN1].bitcast(f8))
                agg = psA.tile([3, GROUP], f32, tag="agg")
                for v in range(VG):
                    cell = g * VG + v
                    k = int(chunks1[cell])
                    cq = int(cbase[cell]) - q0
                    for kk in range(k):
                        nc.tensor.matmul(
                            agg[:, v * SPAN1:(v + 1) * SPAN1],
                            lhsT=sx_t[:, (cq + kk) * 3:(cq + kk) * 3 + 3],
                            rhs=sl_t[:, (cq + kk) * SPAN1:(cq + kk + 1) * SPAN1],
                            start=(kk == 0), stop=(kk == k - 1),
                            skip_group_check=True)
                agg_sb = rows.tile([3, GROUP], f16, tag="aggsb")
                nc.vector.tensor_copy(agg_sb[:], agg[:])
                h1_ps = psB.tile([HID, GROUP], f32, tag="mm")
                nc.tensor.matmul(h1_ps[:], lhsT=W1_t[:], rhs=agg_sb[:],
                                 start=True, stop=True)
                h1_sb = lrelu(rows, h1_ps, b1_t, b1s_t, GROUP, "h1")
                p_ps = psB.tile([HID, GROUP], f32, tag="mm")
                nc.tensor.matmul(p_ps[:], lhsT=W2_t[:], rhs=h1_sb[:],
                                 start=True, stop=True)
                p_sb = rows.tile([HID, GROUP], f16, tag="p")
                nc.vector.tensor_copy(p_sb[:], p_ps[:])
                for tt in range(GROUP // P):
                    tr = psT.tile([P, P], f16, tag="tr")
                    nc.tensor.transpose(tr[:], p_sb[:, tt * P:(tt + 1) * P],
                                        ident[:])
                    r_sb = rows.tile([P, P], f16, tag="rows")
                    nc.scalar.activation(r_sb[:], tr[:], AF.Identity)
                    base = g * GROUP + tt * P
                    nc.sync.dma_start(p_loc[base:base + P, :], r_sb[:])
                    s_sb = rows.tile([P, P], f16, tag="self")
                    nc.vector.tensor_scalar_mul(
                        s_sb[:], r_sb[:],
                        nself_t[:, g * (GROUP // P) + tt:g * (GROUP // P) + tt + 1])
                    nc.sync.dma_start(agg2[base:base + P, :], s_sb[:])

        nc.sync.dma_start(agg2[nloc:nloc + P, :], zrow[:])

        # ---- AllGather p ----
        nc.gpsimd.collective_compute(
            "AllGather", mybir.AluOpType.bypass,
            replica_groups=[list(range(NCORES))],
            ins=[p_loc[:, :].opt()], outs=[p_full[:, :].opt()])

        # ---- conv2: gather -> scale -> scatter-add ----
        maxcols = (BATCH + P - 1) // P
        icols = meta["icols"]
        with ExitStack() as c2:
            idxp = c2.enter_context(tc.tile_pool(name="idxp", bufs=1))
            slabs = c2.enter_context(tc.tile_pool(name="slabs", bufs=3))
            small = c2.enter_context(tc.tile_pool(name="small", bufs=4))
            # idx inputs arrive as 16 partitions (2B/slot); replicate the
            # 16-row block to all 128 partitions on-device (ISA reads the
            # idx AP as 8 replicated 16-partition stripes).
            gIt = idxp.tile([P, icols], mybir.dt.int16)
            sIt = idxp.tile([P, icols], mybir.dt.int16)
            nc.sync.dma_start(gIt[0:16, :], io["gI"][:, :])
            nc.sync.dma_start(sIt[0:16, :], io["sI"][:, :])
            for k in range(1, 8):
                nc.sync.dma_start(gIt[16 * k:16 * (k + 1), :], gIt[0:16, :])
                nc.sync.dma_start(sIt[16 * k:16 * (k + 1), :], sIt[0:16, :])
            qn = 0
            for b in batches:
                nb, io_, no, w = b["nb"], b["io"], b["no"], b["w"]
                cols = (nb + P - 1) // P
                gi = gIt[:, io_:io_ + nb // 16]
                si = sIt[:, io_:io_ + nb // 16]
                nr = small.tile([P, maxcols, 1], f16, tag="nr")
                nc.sync.dma_start(nr[:, :cols, :],
                                  io["nrm"][:, no:no + cols].rearrange(
                                      "p (c o) -> p c o", o=1))
                gat = slabs.tile([P, maxcols, HID], f16, tag="gat")
                nc.gpsimd.dma_gather(
                    out_ap=gat[:, :cols, :],
                    in_ap=p_full[w * nloc:(w + 1) * nloc, :],
                    idxs_ap=gi,
                    num_idxs=nb, num_idxs_reg=nb, elem_size=HID,
                    single_packet=False, queue_num=qn)
                qs = qn
                g_ap = gat[:, :cols, :]
                n_ap = nr[:, :cols, :]
                g_b, n_b = bass.broadcast_tensor_aps(g_ap, n_ap)
                nc.vector.tensor_tensor(g_ap, g_b, n_b,
                                        op=mybir.AluOpType.mult)
                nc.gpsimd.dma_scatter_add(
                    out_ap=agg2[:, :],
                    in_ap=gat[:, :cols, :],
                    idxs_ap=si,
                    num_idxs=nb, num_idxs_reg=nb, elem_size=HID,
                    queue_num=qs)
                qn = 0

        # ---- readout MLP ----
        GT = min(512, bloc)
        ngt = bloc // GT
        with ExitStack() as c3:
            big = c3.enter_context(tc.tile_pool(name="big", bufs=2))
            ro = c3.enter_context(tc.tile_pool(name="ro", bufs=2))
            rps = c3.enter_context(tc.tile_pool(name="rps", bufs=2, space="PSUM"))
            ops = c3.enter_context(tc.tile_pool(name="ops", bufs=2, space="PSUM"))
            y_sb = consts.tile([1, bloc], f32)
            for gt in range(ngt):
                h2T = big.tile([P, GT * NPG], f16, tag="h2T")
                nc.sync.dma_start_transpose(
                    h2T[:], agg2[gt * GT * NPG:(gt + 1) * GT * NPG, :])
                f0 = rps.tile([HID, GT], f32, tag="f")
                for j in range(NPG):
                    zT = h2T[:, j:j + (GT - 1) * NPG + 1:NPG]
                    nc.tensor.matmul(f0[:], lhsT=Wf0_t[:, j, :], rhs=zT,
                                     start=(j == 0), stop=(j == NPG - 1))
                f0s = lrelu(ro, f0, bf0_t, bf0s_t, GT, "f0")
                f1 = rps.tile([HID, GT], f32, tag="f")
                nc.tensor.matmul(f1[:], lhsT=Wf1_t[:], rhs=f0s[:],
                                 start=True, stop=True)
                f1s = lrelu(ro, f1, bf1_t, bf1s_t, GT, "f1")
                o = ops.tile([1, GT], f32, tag="o")
                nc.tensor.matmul(o[:], lhsT=Wout_t[:], rhs=f1s[:],
                                 start=True, stop=True)
                t = ro.tile([1, GT], f32, tag="t")
                nc.scalar.activation(t[:], o[:], AF.Tanh, bias=bo_t[:, 0:1])
                nc.vector.tensor_scalar(y_sb[:, gt * GT:(gt + 1) * GT], t[:],
                                        scalar1=90.0, scalar2=150.0,
                                        op0=mybir.AluOpType.mult,
                                        op1=mybir.AluOpType.add)
            nc.sync.dma_start(y_ap.rearrange("(a b) -> a b", a=1), y_sb[:])


def build_fused(meta):
    bass, bacc, tile, mybir = _bass_mods()
    f16, f32 = mybir.dt.float16, mybir.dt.float32
    i16 = mybir.dt.int16
    nloc = meta["nloc"]
    bloc = nloc // NPG
    nc = bacc.Bacc("TRN2", target_bir_lowering=False, debug=False,
                   num_devices=NCORES, num_swdge_queues=4)
    io = {}
    T1, icols, ncols = meta["T1"], meta["icols"], meta["ncols"]
    i8 = mybir.dt.int8
    specs = [
        ("sx", [P, T1 * 3], i8), ("sel1", [P, T1 * SPAN1], i8),
        ("nself", [P, nloc // P], f32),
        ("gI", [16, icols], i16), ("sI", [16, icols], i16),
        ("nrm", [P, ncols], f16),
        ("W1", [3, HID], f16), ("b1", [HID, 1], f32),
        ("W2", [HID, HID], f16),
        ("Wf0", [HID, NPG * HID], f16), ("bf0", [HID, 1], f32),
        ("Wf1", [HID, HID], f16), ("bf1", [HID, 1], f32),
        ("Wout", [HID, 1], f16), ("bo", [1, 1], f32),
    ]
    for name, shape, dt in specs:
        io[name] = nc.dram_tensor(name, shape, dt, kind="ExternalInput").ap()
    y = nc.dram_tensor("y", [bloc], f32, kind="ExternalOutput").ap()
    with tile.TileContext(nc) as tc:
        _emit(nc, tc, io, meta, y)
    nc.compile()
    return nc


# ----------------------------------------------------------------------------
# SPMD runner (one program, 8 cores, via PJRT shard_map)
# ----------------------------------------------------------------------------

def _make_runner(nc):
    import jax
    from jax.sharding import Mesh, PartitionSpec
    try:
        from jax.experimental.shard_map import shard_map
    except ImportError:
        from jax.shard_map import shard_map
    import concourse.mybir as mybir
    from concourse.bass2jax import (install_neuronx_cc_hook, _bass_exec_p,
                                    partition_id_tensor)
    install_neuronx_cc_hook()
    part_name = nc.partition_id_tensor.name if nc.partition_id_tensor else None
    in_names, out_names, out_avals, zero_shapes = [], [], [], []
    for alloc in nc.m.functions[0].allocations:
        if not isinstance(alloc, mybir.MemoryLocationSet):
            continue
        name = alloc.memorylocations[0].name
        if alloc.kind == "ExternalInput":
            if name != part_name:
                in_names.append(name)
        elif alloc.kind == "ExternalOutput":
            out_names.append(name)
            shape = tuple(alloc.tensor_shape)
            dtype = mybir.dt.np(alloc.dtype)
            out_avals.append(jax.core.ShapedArray(shape, dtype))
            zero_shapes.append((shape, dtype))
    n_params = len(in_names)
    all_in = list(in_names) + list(out_names)
    if part_name is not None:
        all_in = all_in + [part_name]
    donate = tuple(range(n_params, n_params + len(out_names)))

    def _body(*args):
        operands = list(args)
        if part_name is not None:
            operands.append(partition_id_tensor())
        outs = _bass_exec_p.bind(
            *operands,
            out_avals=tuple(out_avals),
            in_names=tuple(all_in),
            out_names=tuple(out_names),
            lowering_input_output_aliases=(),
            sim_require_finite=True,
            sim_require_nnan=True,
            nc=nc,
        )
        return tuple(outs)

    devices = jax.devices()[:NCORES]
    mesh = Mesh(np.asarray(devices), ("core",))
    in_specs = (PartitionSpec("core"),) * (n_params + len(out_names))
    out_specs = (PartitionSpec("core"),) * len(out_names)
    from jax.experimental.shard_map import shard_map as _sm
    jitted = jax.jit(
        _sm(_body, mesh=mesh, in_specs=in_specs, out_specs=out_specs,
            check_rep=False),
        donate_argnums=donate, keep_unused=True)
    return dict(jit=jitted, in_names=in_names, out_names=out_names,
                zero_shapes=zero_shapes, n_params=n_params,
                out_avals=out_avals)


def _concat_inputs(runner, in_maps):
    cat = []
    for name in runner["in_names"]:
        cat.append(np.concatenate([np.ascontiguousarray(m[name])
                                   for m in in_maps], axis=0))
    return cat


def _run_spmd(runner, in_maps):
    import jax
    cat = _concat_inputs(runner, in_maps)
    zeros = [np.zeros((NCORES * s[0], *s[1:]), d)
             for s, d in runner["zero_shapes"]]
    outs = runner["jit"](*cat, *zeros)
    jax.block_until_ready(outs)
    res = []
    for c in range(NCORES):
        res.append({name: np.asarray(outs[i]).reshape(
            NCORES, *runner["out_avals"][i].shape)[c]
            for i, name in enumerate(runner["out_names"])})
    return res


BENCH = False
LAST_TIMINGS = {}
PIPELINE_TIMINGS = {}


def _bench_launch(name, runner, in_maps, iters=3, pipeline_iters=0):
    import time as _time
    import jax
    cat = [jax.device_put(a) for a in _concat_inputs(runner, in_maps)]
    jax.block_until_ready(cat)
    best = None
    for _ in range(iters):
        zeros = [jax.device_put(np.zeros((NCORES * s[0], *s[1:]), d))
                 for s, d in runner["zero_shapes"]]
        jax.block_until_ready(zeros)
        t0 = _time.perf_counter()
        outs = runner["jit"](*cat, *zeros)
        jax.block_until_ready(outs)
        dt = _time.perf_counter() - t0
        best = dt if best is None else min(best, dt)
    LAST_TIMINGS[name] = best
    if pipeline_iters:
        packs = []
        for _ in range(pipeline_iters):
            zeros = [jax.device_put(np.zeros((NCORES * s[0], *s[1:]), d))
                     for s, d in runner["zero_shapes"]]
            packs.append(zeros)
        jax.block_until_ready(packs)
        t0 = _time.perf_counter()
        outs = [runner["jit"](*cat, *z) for z in packs]
        jax.block_until_ready(outs)
        dt = _time.perf_counter() - t0
        PIPELINE_TIMINGS[name] = dt / pipeline_iters


# ----------------------------------------------------------------------------
# top-level kernel
# ----------------------------------------------------------------------------

def kernel(x, edge_index, edge_weight, W1, b1, W2, b2,
           Wf0, bf0, Wf1, bf1, Wout, bout):
    x = np.asarray(x, np.float32)
    src = np.asarray(edge_index[0], np.int64)
    dst = np.asarray(edge_index[1], np.int64)
    ew = np.asarray(edge_weight, np.float32)

    meta, in_maps = _prepare(x, src, dst, ew, NLOC)
    wts = _prep_weights(W1, b1, W2, b2, Wf0, bf0, Wf1, bf1, Wout, bout)
    for m in in_maps:
        m.update(wts)

    nc = build_fused(meta)
    runner = _make_runner(nc)
    res = _run_spmd(runner, in_maps)
    if BENCH:
        _bench_launch("fused", runner, in_maps, pipeline_iters=8)
    y = np.concatenate([r["y"] for r in res]).reshape(B, 1).astype(np.float32)
    return y


# revision 36
# speedup vs baseline: 3.0111x; 1.1539x over previous
# Fused single-launch GCN kernel for Trainium2 (8 NeuronCores, SPMD).
#
# Math (PyG GCNConv x2 + per-graph MLP readout):
#   norm[e] = dinv[src]*ew*dinv[dst]  (dinv = rsqrt(weighted indeg + 1))
#   h1 = leaky_relu(scatter(norm*x[src]) + nself*x[d] @ ... W1 + b1)
#   h2 = scatter(norm*h1[src]) @ W2 + b2  (+ self term)
#   y  = MLP(reshape(h2, [B, 22*128]))
#
# Device plan (ONE launch, SPMD over 8 cores, per-core data via inputs):
#   conv1: host-packed slot payloads (x[src], 3 fp16) + selector strips
#     accumulate agg1[3,512] per dest group via PE chunk matmuls; then
#     h1T = Lrelu(W1^T agg1 + b1); pT = W2^T h1T (W2 folded in BEFORE the
#     exchange, by linearity); PE-transpose -> p rows fp16 -> p_local DRAM;
#     self-loop init: agg2 rows = nself[d] * p[d].
#   AllGather p_local (5.8MB/core -> 46MB) on device.
#   conv2: per source-shard window: dma_gather p rows (int16 idx),
#     scale by per-edge norm (DVE broadcast), dma_scatter_add into agg2
#     (fp16 CCE accumulate). No selector matmuls, no W2 matmul after.
#   readout: dma_start_transpose agg2 -> h2T; per-512-graph-tile MLP
#     (b2 folded into bf0 on host); tanh*90+150 -> y [1024]/core.
#
# Structure metadata (chunk counts, window capacities) is computed at
# runtime from the actual edge data but taken as MAX over cores, so the
# single instruction stream is valid for every core (true SPMD).

import numpy as np

N = 180224
E = 1441792
HID = 128
NPG = 22
NCORES = 8
P = 128
GROUP = 512
SPAN1 = 8
VG = GROUP // SPAN1          # cells (8-dest windows) per group
BATCH = 2048                 # conv2 gather batch (slots)
NLOC = N // NCORES           # 22528
BLOC = NLOC // NPG           # 1024 graphs per core
B = N // NPG


# ----------------------------------------------------------------------------
# host-side structure building
# ----------------------------------------------------------------------------

def _prepare(x, src, dst, ew, nloc):
    """Build per-core input arrays + shared (max-over-cores) structure."""
    n = nloc * NCORES
    ncell = (nloc // GROUP) * VG
    deg = np.bincount(dst, weights=ew, minlength=n).astype(np.float64) + 1.0
    dinv = (1.0 / np.sqrt(deg)).astype(np.float32)
    nself = (1.0 / deg).astype(np.float32)
    norm = (dinv[src] * ew * dinv[dst]).astype(np.float32)

    order = np.argsort(dst, kind="stable")
    so, do_, no_ = src[order], dst[order], norm[order]
    bounds = np.searchsorted(do_, np.arange(NCORES + 1) * nloc)

    percore = []
    cnt1 = np.zeros((NCORES, ncell), np.int64)
    cnt2 = np.zeros((NCORES, NCORES), np.int64)
    for c in range(NCORES):
        e0, e1 = bounds[c], bounds[c + 1]
        s2, d2, v2 = so[e0:e1], do_[e0:e1] - c * nloc, no_[e0:e1]
        # conv1 includes self loops as regular slots
        s1 = np.concatenate([s2, np.arange(nloc, dtype=np.int64) + c * nloc])
        d1 = np.concatenate([d2, np.arange(nloc, dtype=np.int64)])
        v1 = np.concatenate([v2, nself[c * nloc:(c + 1) * nloc]])
        cell = d1 // SPAN1
        cnt1[c] = np.bincount(cell, minlength=ncell)
        w2 = s2 // nloc
        cnt2[c] = np.bincount(w2, minlength=NCORES)
        percore.append((s1, d1, v1, cell, s2, d2, v2, w2))

    cap1 = cnt1.max(0)
    chunks1 = (cap1 + P - 1) // P            # >=1 (self loops)
    cbase = np.concatenate([[0], np.cumsum(chunks1)]).astype(np.int64)
    T1 = int(cbase[-1])

    # conv2 rounds: within each source window, slots are split by their
    # occurrence rank per destination, so every scatter-add instruction
    # has UNIQUE destination indices (HW CCE races on duplicates).
    rankmax = 0
    ranks_pc = []
    for c in range(NCORES):
        s2, d2, w2 = percore[c][4], percore[c][5], percore[c][7]
        o2 = np.lexsort((d2, w2))
        d2o, w2o = d2[o2], w2[o2]
        key = w2o * nloc + d2o
        # occurrence rank within (w, dst)
        newrun = np.concatenate([[True], key[1:] != key[:-1]])
        runid = np.cumsum(newrun) - 1
        runstart = np.flatnonzero(newrun)
        rank = np.arange(len(key)) - runstart[runid]
        ranks_pc.append((o2, rank))
        if len(rank):
            rankmax = max(rankmax, int(rank.max()) + 1)
    # counts per (window, round)
    cnt3 = np.zeros((NCORES, NCORES, rankmax), np.int64)
    for c in range(NCORES):
        o2, rank = ranks_pc[c]
        w2o = percore[c][7][o2]
        np.add.at(cnt3[c], (w2o, rank), 1)
    capwr = ((cnt3.max(0) + P - 1) // P * P).astype(np.int64)  # [8, rankmax]
    batches = []
    icols = ncols = 0
    soff = 0
    slotbase = np.zeros((NCORES, rankmax), np.int64)
    for w in range(NCORES):
        for r in range(rankmax):
            if capwr[w, r] == 0:
                continue
            slotbase[w, r] = soff
            off = 0
            while off < capwr[w, r]:
                nb = int(min(BATCH, capwr[w, r] - off))
                batches.append(dict(w=w, nb=nb, io=icols, no=ncols,
                                    so=soff + off))
                icols += nb // 16
                ncols += (nb + P - 1) // P
                off += nb
            soff += int(capwr[w, r])
    TS = soff
    meta = dict(chunks1=chunks1, cbase=cbase, T1=T1, capwr=capwr,
                batches=batches, icols=icols, ncols=ncols, nloc=nloc)

    in_maps = []
    for c in range(NCORES):
        s1, d1, v1, cell, s2, d2, v2, w2 = percore[c]
        o = np.argsort(cell, kind="stable")
        s1o, d1o, v1o, co = s1[o], d1[o], v1[o], cell[o]
        cstart = np.concatenate([[0], np.cumsum(cnt1[c])])
        rank = np.arange(len(co)) - cstart[co]
        slot = cbase[co] * P + rank
        import ml_dtypes
        f8 = ml_dtypes.float8_e4m3
        sxf = np.zeros((T1 * P, 3), f8)
        sxf[slot] = x[s1o].astype(f8)
        sx = np.ascontiguousarray(
            sxf.reshape(T1, P, 3).transpose(1, 0, 2)).reshape(P, T1 * 3)
        sel = np.zeros((P, T1 * SPAN1), f8)
        chunk = cbase[co] + rank // P
        selcol = chunk * SPAN1 + (d1o - co * SPAN1)
        sel[rank % P, selcol] = v1o.astype(f8)
        # ship fp8 bytes as int8 (XLA on TRN2 rejects fp8 dtypes)
        sx = sx.view(np.int8)
        sel = sel.view(np.int8)
        nst = np.ascontiguousarray(
            nself[c * nloc:(c + 1) * nloc].reshape(nloc // P, P).T)

        o2, rank = ranks_pc[c]
        s2o, d2o, v2o, w2o = s2[o2], d2[o2], v2[o2], w2[o2]
        cell2 = w2o * rankmax + rank
        sb_flat = slotbase.reshape(-1)
        p2 = np.argsort(cell2, kind="stable")
        c2s = cell2[p2]
        nr2 = np.concatenate([[True], c2s[1:] != c2s[:-1]])
        rid = np.cumsum(nr2) - 1
        rstart = np.flatnonzero(nr2)
        within = np.arange(len(c2s)) - rstart[rid]
        slot2 = np.empty(len(c2s), np.int64)
        slot2[p2] = sb_flat[c2s] + within
        gidx = np.zeros(TS, np.int16)
        gidx[slot2] = (s2o - w2o * nloc).astype(np.int16)
        sidx = np.full(TS, nloc, np.int16)   # dummy row (norm=0 slots)
        sidx[slot2] = d2o.astype(np.int16)
        nrm = np.zeros(TS, np.float16)
        nrm[slot2] = v2o.astype(np.float16)

        gI = np.zeros((16, icols), np.int16)
        sI = np.zeros((16, icols), np.int16)
        NR = np.zeros((P, ncols), np.float16)
        for b in batches:
            nb, io, no, sof = b["nb"], b["io"], b["no"], b["so"]
            gI[:, io:io + nb // 16] = gidx[sof:sof + nb].reshape(nb // 16, 16).T
            sI[:, io:io + nb // 16] = sidx[sof:sof + nb].reshape(nb // 16, 16).T
            cols = (nb + P - 1) // P
            nrb = np.zeros(cols * P, np.float16)
            nrb[:nb] = nrm[sof:sof + nb]
            NR[:, no:no + cols] = nrb.reshape(cols, P).T
        in_maps.append(dict(sx=sx, sel1=sel, nself=nst, gI=gI, sI=sI, nrm=NR))
    return meta, in_maps


def _prep_weights(W1, b1, W2, b2, Wf0, bf0, Wf1, bf1, Wout, bout):
    W1 = np.asarray(W1, np.float32)
    b2 = np.asarray(b2, np.float32).reshape(-1)
    Wf0 = np.asarray(Wf0, np.float32)
    Wf0r = np.ascontiguousarray(
        Wf0.reshape(NPG, HID, HID).transpose(1, 0, 2)).reshape(HID, NPG * HID)
    bf0p = np.asarray(bf0, np.float32).reshape(-1) + np.tile(b2, NPG) @ Wf0
    return dict(
        W1=W1.astype(np.float16),
        b1=np.asarray(b1, np.float32).reshape(HID, 1),
        W2=np.asarray(W2, np.float16),
        Wf0=Wf0r.astype(np.float16),
        bf0=bf0p.astype(np.float32).reshape(HID, 1),
        Wf1=np.asarray(Wf1, np.float16),
        bf1=np.asarray(bf1, np.float32).reshape(HID, 1),
        Wout=np.asarray(Wout, np.float32).astype(np.float16).reshape(HID, 1),
        bo=np.asarray(bout, np.float32).reshape(1, 1),
    )


# ----------------------------------------------------------------------------
# device program
# ----------------------------------------------------------------------------

def _bass_mods():
    import concourse.bass as bass
    import concourse.bacc as bacc
    import concourse.tile as tile
    from concourse import mybir
    return bass, bacc, tile, mybir


def _emit(nc, tc, io, meta, y_ap):
    """Emit the fused program. io: dict name->AP of ExternalInputs."""
    bass, bacc, tile, mybir = _bass_mods()
    from concourse.masks import make_identity
    from contextlib import ExitStack

    f16, f32 = mybir.dt.float16, mybir.dt.float32
    nloc = meta["nloc"]
    ng = nloc // GROUP
    chunks1, cbase, T1 = meta["chunks1"], meta["cbase"], meta["T1"]
    batches = meta["batches"]
    bloc = nloc // NPG
    AF = mybir.ActivationFunctionType

    with ExitStack() as ctx:
        consts = ctx.enter_context(tc.tile_pool(name="consts", bufs=1))
        dram = ctx.enter_context(tc.tile_pool(name="dram", bufs=1, space="DRAM"))

        W1_t = consts.tile([3, HID], f16)
        nc.sync.dma_start(W1_t[:], io["W1"][:])
        b1_t = consts.tile([HID, 1], f32)
        nc.sync.dma_start(b1_t[:], io["b1"][:])
        W2_t = consts.tile([HID, HID], f16)
        nc.sync.dma_start(W2_t[:], io["W2"][:])
        nself_t = consts.tile([P, nloc // P], f32)
        nc.sync.dma_start(nself_t[:], io["nself"][:])
        Wf0_t = consts.tile([HID, NPG, HID], f16)
        nc.sync.dma_start(Wf0_t[:], io["Wf0"].rearrange("k (j m) -> k j m", j=NPG))
        bf0_t = consts.tile([HID, 1], f32)
        nc.sync.dma_start(bf0_t[:], io["bf0"][:])
        Wf1_t = consts.tile([HID, HID], f16)
        nc.sync.dma_start(Wf1_t[:], io["Wf1"][:])
        bf1_t = consts.tile([HID, 1], f32)
        nc.sync.dma_start(bf1_t[:], io["bf1"][:])
        Wout_t = consts.tile([HID, 1], f16)
        nc.sync.dma_start(Wout_t[:], io["Wout"][:])
        bo_t = consts.tile([1, 1], f32)
        nc.sync.dma_start(bo_t[:], io["bo"][:])
        ident = consts.tile([P, P], f16)
        make_identity(nc, ident)
        b1s_t = consts.tile([HID, 1], f32)
        nc.vector.tensor_scalar_mul(b1s_t[:], b1_t[:], 0.01)
        bf0s_t = consts.tile([HID, 1], f32)
        nc.vector.tensor_scalar_mul(bf0s_t[:], bf0_t[:], 0.01)
        bf1s_t = consts.tile([HID, 1], f32)
        nc.vector.tensor_scalar_mul(bf1s_t[:], bf1_t[:], 0.01)

        def lrelu(pool, ps, bias, bias_s, w, tag):
            a_t = pool.tile([HID, w], f32, tag=tag + "a")
            nc.scalar.activation(a_t[:], ps[:], AF.Identity,
                                 bias=bias[:, 0:1])
            c_t = pool.tile([HID, w], f32, tag=tag + "b")
            nc.scalar.activation(c_t[:], ps[:], AF.Identity,
                                 bias=bias_s[:, 0:1], scale=0.01)
            m_t = pool.tile([HID, w], f16, tag=tag + "m")
            nc.vector.tensor_tensor(m_t[:], a_t[:], c_t[:],
                                    op=mybir.AluOpType.max)
            return m_t

        zrow = consts.tile([P, HID], f16)
        nc.vector.memset(zrow[:], 0.0)

        p_loc = dram.tile([nloc, HID], f16)
        p_full = dram.tile([nloc * NCORES, HID], f16)
        agg2 = dram.tile([nloc + P, HID], f16)   # +dummy rows for pad slots

        # ---- conv1 + p = h1@W2 + self-loop init of agg2 ----
        gch = [int(cbase[(g + 1) * VG] - cbase[g * VG]) for g in range(ng)]
        max_gch = max(gch)
        with ExitStack() as c1:
            sb = c1.enter_context(tc.tile_pool(name="sb", bufs=3))
            rows = c1.enter_context(tc.tile_pool(name="rows", bufs=3))
            psA = c1.enter_context(tc.tile_pool(name="psA", bufs=2, space="PSUM"))
            psB = c1.enter_context(tc.tile_pool(name="psB", bufs=2, space="PSUM"))
            psT = c1.enter_context(tc.tile_pool(name="psT", bufs=2, space="PSUM"))
            for g in range(ng):
                q0 = int(cbase[g * VG])
                gc = gch[g]
                f8 = mybir.dt.float8e4
                sx_t = sb.tile([P, max_gch * 3], f8, tag="sx")
                nc.sync.dma_start(sx_t[:, :gc * 3],
                                  io["sx"][:, q0 * 3:(q0 + gc) * 3].bitcast(f8))
                sl_t = sb.tile([P, max_gch * SPAN1], f8, tag="sel")
                nc.sync.dma_start(
                    sl_t[:, :gc * SPAN1],
                    io["sel1"][:, q0 * SPAN1:(q0 + gc) * SPAN1].bitcast(f8))
                agg = psA.tile([3, GROUP], f32, tag="agg")
                for v in range(VG):
                    cell = g * VG + v
                    k = int(chunks1[cell])
                    cq = int(cbase[cell]) - q0
                    for kk in range(k):
                        nc.tensor.matmul(
                            agg[:, v * SPAN1:(v + 1) * SPAN1],
                            lhsT=sx_t[:, (cq + kk) * 3:(cq + kk) * 3 + 3],
                            rhs=sl_t[:, (cq + kk) * SPAN1:(cq + kk + 1) * SPAN1],
                            start=(kk == 0), stop=(kk == k - 1),
                            skip_group_check=True)
                agg_sb = rows.tile([3, GROUP], f16, tag="aggsb")
                nc.vector.tensor_copy(agg_sb[:], agg[:])
                h1_ps = psB.tile([HID, GROUP], f32, tag="mm")
                nc.tensor.matmul(h1_ps[:], lhsT=W1_t[:], rhs=agg_sb[:],
                                 start=True, stop=True)
                h1_sb = lrelu(rows, h1_ps, b1_t, b1s_t, GROUP, "h1")
                p_ps = psB.tile([HID, GROUP], f32, tag="mm")
                nc.tensor.matmul(p_ps[:], lhsT=W2_t[:], rhs=h1_sb[:],
                                 start=True, stop=True)
                p_sb = rows.tile([HID, GROUP], f16, tag="p")
                nc.vector.tensor_copy(p_sb[:], p_ps[:])
                for tt in range(GROUP // P):
                    tr = psT.tile([P, P], f16, tag="tr")
                    nc.tensor.transpose(tr[:], p_sb[:, tt * P:(tt + 1) * P],
                                        ident[:])
                    r_sb = rows.tile([P, P], f16, tag="rows")
                    nc.scalar.activation(r_sb[:], tr[:], AF.Identity)
                    base = g * GROUP + tt * P
                    nc.sync.dma_start(p_loc[base:base + P, :], r_sb[:])
                    s_sb = rows.tile([P, P], f16, tag="self")
                    nc.vector.tensor_scalar_mul(
                        s_sb[:], r_sb[:],
                        nself_t[:, g * (GROUP // P) + tt:g * (GROUP // P) + tt + 1])
                    nc.sync.dma_start(agg2[base:base + P, :], s_sb[:])

        nc.sync.dma_start(agg2[nloc:nloc + P, :], zrow[:])

        # ---- AllGather p ----
        nc.gpsimd.collective_compute(
            "AllGather", mybir.AluOpType.bypass,
            replica_groups=[list(range(NCORES))],
            ins=[p_loc[:, :].opt()], outs=[p_full[:, :].opt()])

        # ---- conv2: gather -> scale -> scatter-add ----
        maxcols = (BATCH + P - 1) // P
        icols = meta["icols"]
        with ExitStack() as c2:
            idxp = c2.enter_context(tc.tile_pool(name="idxp", bufs=1))
            slabs = c2.enter_context(tc.tile_pool(name="slabs", bufs=3))
            small = c2.enter_context(tc.tile_pool(name="small", bufs=4))
            # idx inputs arrive as 16 partitions (2B/slot); replicate the
            # 16-row block to all 128 partitions on-device (ISA reads the
            # idx AP as 8 replicated 16-partition stripes).
            gIt = idxp.tile([P, icols], mybir.dt.int16)
            sIt = idxp.tile([P, icols], mybir.dt.int16)
            nc.sync.dma_start(gIt[0:16, :], io["gI"][:, :])
            nc.sync.dma_start(sIt[0:16, :], io["sI"][:, :])
            for k in range(1, 8):
                nc.sync.dma_start(gIt[16 * k:16 * (k + 1), :], gIt[0:16, :])
                nc.sync.dma_start(sIt[16 * k:16 * (k + 1), :], sIt[0:16, :])
            qn = 0
            for b in batches:
                nb, io_, no, w = b["nb"], b["io"], b["no"], b["w"]
                cols = (nb + P - 1) // P
                gi = gIt[:, io_:io_ + nb // 16]
                si = sIt[:, io_:io_ + nb // 16]
                nr = small.tile([P, maxcols, 1], f16, tag="nr")
                nc.sync.dma_start(nr[:, :cols, :],
                                  io["nrm"][:, no:no + cols].rearrange(
                                      "p (c o) -> p c o", o=1))
                gat = slabs.tile([P, maxcols, HID], f16, tag="gat")
                nc.gpsimd.dma_gather(
                    out_ap=gat[:, :cols, :],
                    in_ap=p_full[w * nloc:(w + 1) * nloc, :],
                    idxs_ap=gi,
                    num_idxs=nb, num_idxs_reg=nb, elem_size=HID,
                    single_packet=False, queue_num=qn)
                qs = qn
                g_ap = gat[:, :cols, :]
                n_ap = nr[:, :cols, :]
                g_b, n_b = bass.broadcast_tensor_aps(g_ap, n_ap)
                nc.vector.tensor_tensor(g_ap, g_b, n_b,
                                        op=mybir.AluOpType.mult)
                nc.gpsimd.dma_scatter_add(
                    out_ap=agg2[:, :],
                    in_ap=gat[:, :cols, :],
                    idxs_ap=si,
                    num_idxs=nb, num_idxs_reg=nb, elem_size=HID,
                    queue_num=qs)
                qn = 0

        # ---- readout MLP ----
        GT = min(512, bloc)
        ngt = bloc // GT
        with ExitStack() as c3:
            big = c3.enter_context(tc.tile_pool(name="big", bufs=2))
            ro = c3.enter_context(tc.tile_pool(name="ro", bufs=2))
            rps = c3.enter_context(tc.tile_pool(name="rps", bufs=2, space="PSUM"))
            ops = c3.enter_context(tc.tile_pool(name="ops", bufs=2, space="PSUM"))
            y_sb = consts.tile([1, bloc], f32)
            for gt in range(ngt):
                h2T = big.tile([P, GT * NPG], f16, tag="h2T")
                nc.sync.dma_start_transpose(
                    h2T[:], agg2[gt * GT * NPG:(gt + 1) * GT * NPG, :])
                f0 = rps.tile([HID, GT], f32, tag="f")
                for j in range(NPG):
                    zT = h2T[:, j:j + (GT - 1) * NPG + 1:NPG]
                    nc.tensor.matmul(f0[:], lhsT=Wf0_t[:, j, :], rhs=zT,
                                     start=(j == 0), stop=(j == NPG - 1))
                f0s = lrelu(ro, f0, bf0_t, bf0s_t, GT, "f0")
                f1 = rps.tile([HID, GT], f32, tag="f")
                nc.tensor.matmul(f1[:], lhsT=Wf1_t[:], rhs=f0s[:],
                                 start=True, stop=True)
                f1s = lrelu(ro, f1, bf1_t, bf1s_t, GT, "f1")
                o = ops.tile([1, GT], f32, tag="o")
                nc.tensor.matmul(o[:], lhsT=Wout_t[:], rhs=f1s[:],
                                 start=True, stop=True)
                t = ro.tile([1, GT], f32, tag="t")
                nc.scalar.activation(t[:], o[:], AF.Tanh, bias=bo_t[:, 0:1])
                nc.vector.tensor_scalar(y_sb[:, gt * GT:(gt + 1) * GT], t[:],
                                        scalar1=90.0, scalar2=150.0,
                                        op0=mybir.AluOpType.mult,
                                        op1=mybir.AluOpType.add)
            nc.sync.dma_start(y_ap.rearrange("(a b) -> a b", a=1), y_sb[:])


def build_fused(meta):
    bass, bacc, tile, mybir = _bass_mods()
    f16, f32 = mybir.dt.float16, mybir.dt.float32
    i16 = mybir.dt.int16
    nloc = meta["nloc"]
    bloc = nloc // NPG
    nc = bacc.Bacc("TRN2", target_bir_lowering=False, debug=False,
                   num_devices=NCORES, num_swdge_queues=4)
    io = {}
    T1, icols, ncols = meta["T1"], meta["icols"], meta["ncols"]
    i8 = mybir.dt.int8
    specs = [
        ("sx", [P, T1 * 3], i8), ("sel1", [P, T1 * SPAN1], i8),
        ("nself", [P, nloc // P], f32),
        ("gI", [16, icols], i16), ("sI", [16, icols], i16),
        ("nrm", [P, ncols], f16),
        ("W1", [3, HID], f16), ("b1", [HID, 1], f32),
        ("W2", [HID, HID], f16),
        ("Wf0", [HID, NPG * HID], f16), ("bf0", [HID, 1], f32),
        ("Wf1", [HID, HID], f16), ("bf1", [HID, 1], f32),
        ("Wout", [HID, 1], f16), ("bo", [1, 1], f32),
    ]
    for name, shape, dt in specs:
        io[name] = nc.dram_tensor(name, shape, dt, kind="ExternalInput").ap()
    y = nc.dram_tensor("y", [bloc], f32, kind="ExternalOutput").ap()
    with tile.TileContext(nc) as tc:
        _emit(nc, tc, io, meta, y)
    nc.compile()
    return nc


# ----------------------------------------------------------------------------
# SPMD runner (one program, 8 cores, via PJRT shard_map)
# ----------------------------------------------------------------------------

def _make_runner(nc):
    import jax
    from jax.sharding import Mesh, PartitionSpec
    try:
        from jax.experimental.shard_map import shard_map
    except ImportError:
        from jax.shard_map import shard_map
    import concourse.mybir as mybir
    from concourse.bass2jax import (install_neuronx_cc_hook, _bass_exec_p,
                                    partition_id_tensor)
    install_neuronx_cc_hook()
    part_name = nc.partition_id_tensor.name if nc.partition_id_tensor else None
    in_names, out_names, out_avals, zero_shapes = [], [], [], []
    for alloc in nc.m.functions[0].allocations:
        if not isinstance(alloc, mybir.MemoryLocationSet):
            continue
        name = alloc.memorylocations[0].name
        if alloc.kind == "ExternalInput":
            if name != part_name:
                in_names.append(name)
        elif alloc.kind == "ExternalOutput":
            out_names.append(name)
            shape = tuple(alloc.tensor_shape)
            dtype = mybir.dt.np(alloc.dtype)
            out_avals.append(jax.core.ShapedArray(shape, dtype))
            zero_shapes.append((shape, dtype))
    n_params = len(in_names)
    all_in = list(in_names) + list(out_names)
    if part_name is not None:
        all_in = all_in + [part_name]
    donate = tuple(range(n_params, n_params + len(out_names)))

    def _body(*args):
        operands = list(args)
        if part_name is not None:
            operands.append(partition_id_tensor())
        outs = _bass_exec_p.bind(
            *operands,
            out_avals=tuple(out_avals),
            in_names=tuple(all_in),
            out_names=tuple(out_names),
            lowering_input_output_aliases=(),
            sim_require_finite=True,
            sim_require_nnan=True,
            nc=nc,
        )
        return tuple(outs)

    devices = jax.devices()[:NCORES]
    mesh = Mesh(np.asarray(devices), ("core",))
    in_specs = (PartitionSpec("core"),) * (n_params + len(out_names))
    out_specs = (PartitionSpec("core"),) * len(out_names)
    from jax.experimental.shard_map import shard_map as _sm
    jitted = jax.jit(
        _sm(_body, mesh=mesh, in_specs=in_specs, out_specs=out_specs,
            check_rep=False),
        donate_argnums=donate, keep_unused=True)
    return dict(jit=jitted, in_names=in_names, out_names=out_names,
                zero_shapes=zero_shapes, n_params=n_params,
                out_avals=out_avals)


def _concat_inputs(runner, in_maps):
    cat = []
    for name in runner["in_names"]:
        cat.append(np.concatenate([np.ascontiguousarray(m[name])
                                   for m in in_maps], axis=0))
    return cat


def _run_spmd(runner, in_maps):
    import jax
    cat = _concat_inputs(runner, in_maps)
    zeros = [np.zeros((NCORES * s[0], *s[1:]), d)
             for s, d in runner["zero_shapes"]]
    outs = runner["jit"](*cat, *zeros)
    jax.block_until_ready(outs)
    res = []
    for c in range(NCORES):
        res.append({name: np.asarray(outs[i]).reshape(
            NCORES, *runner["out_avals"][i].shape)[c]
            for i, name in enumerate(runner["out_names"])})
    return res


BENCH = False
LAST_TIMINGS = {}
PIPELINE_TIMINGS = {}


def _bench_launch(name, runner, in_maps, iters=5, pipeline_iters=0):
    import time as _time
    import jax
    from jax.sharding import Mesh, PartitionSpec, NamedSharding
    mesh = Mesh(np.asarray(jax.devices()[:NCORES]), ("core",))
    sh = NamedSharding(mesh, PartitionSpec("core"))
    cat = [jax.device_put(a, sh) for a in _concat_inputs(runner, in_maps)]
    jax.block_until_ready(cat)
    best = None
    for _ in range(iters):
        zeros = [jax.device_put(np.zeros((NCORES * s[0], *s[1:]), d), sh)
                 for s, d in runner["zero_shapes"]]
        jax.block_until_ready(zeros)
        t0 = _time.perf_counter()
        outs = runner["jit"](*cat, *zeros)
        jax.block_until_ready(outs)
        dt = _time.perf_counter() - t0
        best = dt if best is None else min(best, dt)
    LAST_TIMINGS[name] = best
    if pipeline_iters:
        packs = []
        for _ in range(pipeline_iters):
            zeros = [jax.device_put(np.zeros((NCORES * s[0], *s[1:]), d), sh)
                     for s, d in runner["zero_shapes"]]
            packs.append(zeros)
        jax.block_until_ready(packs)
        t0 = _time.perf_counter()
        outs = [runner["jit"](*cat, *z) for z in packs]
        jax.block_until_ready(outs)
        dt = _time.perf_counter() - t0
        PIPELINE_TIMINGS[name] = dt / pipeline_iters


# ----------------------------------------------------------------------------
# top-level kernel
# ----------------------------------------------------------------------------

def kernel(x, edge_index, edge_weight, W1, b1, W2, b2,
           Wf0, bf0, Wf1, bf1, Wout, bout):
    x = np.asarray(x, np.float32)
    src = np.asarray(edge_index[0], np.int64)
    dst = np.asarray(edge_index[1], np.int64)
    ew = np.asarray(edge_weight, np.float32)

    meta, in_maps = _prepare(x, src, dst, ew, NLOC)
    wts = _prep_weights(W1, b1, W2, b2, Wf0, bf0, Wf1, bf1, Wout, bout)
    for m in in_maps:
        m.update(wts)

    nc = build_fused(meta)
    runner = _make_runner(nc)
    res = _run_spmd(runner, in_maps)
    if BENCH:
        _bench_launch("fused", runner, in_maps, pipeline_iters=8)
    y = np.concatenate([r["y"] for r in res]).reshape(B, 1).astype(np.float32)
    return y
